# revision 1
# baseline (speedup 1.0000x reference)
"""Two-layer GAT (single-head, PyG-style) on 8 Trainium2 NeuronCores.

Strategy (graph/data parallel, destination-sharded):
  - Destination nodes are partitioned into 8 contiguous ranges (one per
    core). Within each core's shard, nodes are sorted by in-degree so
    blocks of 128 nodes have near-uniform degree; each block gets a fixed
    per-node slot count K_b (max over the 8 cores, so the SPMD program is
    identical on all cores). Edge -> slot assignment and all index tables
    are built on the host; padding slots point at a guard row whose
    attention logit is -1e9 (=> softmax weight exp(..) == 0).
  - Each core computes h_aug = x_shard @ [W | W@a_src | W@a_dst] for its
    own nodes (PE), an 8-core AllGather replicates the full augmented
    feature table (rows padded to 512B), and per-edge source rows are
    fetched with the SWDGE dma_gather instruction (one instruction per
    ~64-slot super-block, thousands of rows each). The int16 index limit
    is dodged by biasing: the source AP points at row 32768 and indices
    are signed offsets (the Q7 descriptor generator does a signed
    multiply-accumulate), covering all ~50k rows.
  - Attention softmax runs on DVE/ACT without the max-subtraction
    (logits are O(10); exp is safe in fp32 and the result is identical),
    the weighted segment-sum is a strided DVE reduce over each node's
    slots, and z arrives free as the Exp activation's accumulator.
  - Same structure for layer 2, then row L2-normalization (exp(-0.5 ln)
    to stay inside one ACT table) and a final host-side unpermutation.
"""

import numpy as np

import concourse.bacc as bacc
import concourse.bass as bass
import concourse.mybir as mybir
import concourse.tile as tile
from concourse import bass_utils, library_config

F32 = mybir.dt.float32
I32 = mybir.dt.int32
P = 128
ROWE = 128          # table row = 128 fp32 elements = 512B
SLOPE = 0.2
NEG_BIG = -1.0e9

# Set by kernel() after each run so test.py can report HW exec time.
LAST_RESULTS = None
LAST_META = None

# (key -> (meta, nc)) so repeated calls with the same graph skip the
# multi-minute neuronxcc compile.
_PROGRAM_CACHE = {}


# --------------------------------------------------------------------------
# Host-side preprocessing: permutation, slot tables, super-block grouping
# --------------------------------------------------------------------------

def _host_prep(src, dst, n_nodes, n_cores, slot_budget):
    assert n_nodes % n_cores == 0
    shard_orig = n_nodes // n_cores
    # at least one ghost row per shard: ghost rows double as the gather
    # guard (h=0, s=-1e9), and they are filled by the AllGather itself
    nblk = -(-(shard_orig + 1) // P)
    shard = nblk * P
    np_tot = n_cores * shard
    guard = shard_orig  # first ghost row of core 0's shard (s = -1e9)

    deg = np.bincount(dst, minlength=n_nodes)

    perm_new2old = np.full(n_cores * shard, -1, np.int64)
    perm_old2new = np.empty(n_nodes, np.int64)
    Kb_per_core = np.zeros((n_cores, nblk), np.int64)
    for c in range(n_cores):
        ids = np.arange(c * shard_orig, (c + 1) * shard_orig)
        order = np.argsort(-deg[ids], kind="stable")
        sorted_ids = ids[order]
        perm_new2old[c * shard : c * shard + shard_orig] = sorted_ids
        perm_old2new[sorted_ids] = c * shard + np.arange(shard_orig)
        dsort = np.concatenate(
            [deg[sorted_ids], np.zeros(shard - shard_orig, np.int64)]
        )
        Kb_per_core[c] = dsort.reshape(nblk, P).max(axis=1)
    Kb = np.maximum(Kb_per_core.max(axis=0), 1)

    # group blocks into super-blocks of at most slot_budget slots
    supers = []
    b = 0
    while b < nblk:
        s = int(Kb[b])
        nb = 1
        while b + nb < nblk and s + Kb[b + nb] <= slot_budget:
            s += int(Kb[b + nb])
            nb += 1
        supers.append((b, nb, s))
        b += nb

    # dense slot table in permuted space: [np_tot, Kmax]
    kmax = int(Kb.max())
    new_dst = perm_old2new[dst]
    new_src = perm_old2new[src]
    order = np.argsort(new_dst, kind="stable")
    sdst = new_dst[order]
    ssrc = new_src[order]
    counts = np.bincount(sdst, minlength=np_tot)
    starts = np.concatenate([[0], np.cumsum(counts)[:-1]])
    slot_of_edge = np.arange(len(sdst)) - starts[sdst]
    table = np.full((np_tot, kmax), guard, np.int64)
    table[sdst, slot_of_edge] = ssrc

    # per-core slot tables as one [128, sum(Kb)] image (block b occupies
    # columns [cum_b, cum_b + K_b)), flattened row-major
    idx_chunks = []
    for c in range(n_cores):
        img = np.concatenate(
            [
                table[c * shard + b * P : c * shard + (b + 1) * P, : Kb[b]]
                for b in range(nblk)
            ],
            axis=1,
        ).astype(np.int32)
        idx_chunks.append(np.ascontiguousarray(img).reshape(-1))
    idx_flat = np.stack(idx_chunks)

    return dict(
        perm_new2old=perm_new2old,
        perm_old2new=perm_old2new,
        shard=shard,
        shard_orig=shard_orig,
        nblk=nblk,
        np_tot=np_tot,
        guard=guard,
        Kb=Kb,
        supers=supers,
        idx_flat=idx_flat,
    )


# --------------------------------------------------------------------------
# Device program
# --------------------------------------------------------------------------

def _build_nc(
    meta,
    n_cores,
    in_dim,
    hid,
    out_dim,
    bench_reps=0,
    hg_shared=True,
    preload_idx=True,
    gather_bufs=3,
):
    """Build the complete Bass program. Same program runs on every core.

    bench_reps > 0 wraps each aggregation phase in a For_i loop executing
    it bench_reps times (idempotent), for wall-clock-differenced timing.
    """
    shard = meta["shard"]
    nblk = meta["nblk"]
    np_tot = meta["np_tot"]
    Kb = meta["Kb"]
    shard_orig = meta["shard_orig"]
    tot_idx = meta["idx_flat"].shape[1]
    augc = hid + 2  # h | s | d

    nc = bacc.Bacc(
        "TRN2", target_bir_lowering=False, debug=False, num_devices=n_cores
    )

    xin = nc.dram_tensor("x_shard", [shard, in_dim], F32, kind="ExternalInput").ap()
    idx_in = nc.dram_tensor("idx", [tot_idx], I32, kind="ExternalInput").ap()
    gmask_in = nc.dram_tensor("gmask", [P, nblk], F32, kind="ExternalInput").ap()
    ident_in = nc.dram_tensor("ident", [P, P], F32, kind="ExternalInput").ap()
    w1_in = nc.dram_tensor("W1", [in_dim, hid], F32, kind="ExternalInput").ap()
    a1s_in = nc.dram_tensor("a1s", [hid, 1], F32, kind="ExternalInput").ap()
    a1d_in = nc.dram_tensor("a1d", [hid, 1], F32, kind="ExternalInput").ap()
    b1_in = nc.dram_tensor("b1", [1, hid], F32, kind="ExternalInput").ap()
    w2_in = nc.dram_tensor("W2", [hid, out_dim], F32, kind="ExternalInput").ap()
    a2s_in = nc.dram_tensor("a2s", [out_dim, 1], F32, kind="ExternalInput").ap()
    a2d_in = nc.dram_tensor("a2d", [out_dim, 1], F32, kind="ExternalInput").ap()
    b2_in = nc.dram_tensor("b2", [1, out_dim], F32, kind="ExternalInput").ap()
    out_t = nc.dram_tensor("out", [shard, out_dim], F32, kind="ExternalOutput").ap()

    groups = [list(range(n_cores))]
    shared = "Shared" if (n_cores > 4 and hg_shared) else "Local"

    with tile.TileContext(nc) as tc:
        with (
            tc.tile_pool(name="dram", bufs=1, space="DRAM") as dram,
            tc.tile_pool(name="consts", bufs=1) as consts,
            tc.tile_pool(name="sb", bufs=3) as sb,
            tc.tile_pool(name="gather", bufs=gather_bufs) as gp,
            tc.tile_pool(name="psum", bufs=2, space="PSUM") as pp,
        ):
            sh1 = dram.tile([shard, ROWE], F32)
            hg1 = dram.tile([np_tot, ROWE], F32, addr_space=shared)
            sh2 = dram.tile([shard, ROWE], F32)
            hg2 = dram.tile([np_tot, ROWE], F32, addr_space=shared)

            # ---- constants / setup ------------------------------------
            identity = consts.tile([P, P], F32)
            nc.sync.dma_start(out=identity[:], in_=ident_in)
            gmask = consts.tile([P, nblk], F32)
            nc.sync.dma_start(out=gmask[:], in_=gmask_in)

            def ghost_fix(h_sb, blk):
                """Push ghost rows' logit to -1e9 so they act as gather
                guards: s += gmask * (-1e9). Full-partition op (the BIR
                verifier rejects partition-offset starts)."""
                if (blk + 1) * P > shard_orig:
                    nc.vector.scalar_tensor_tensor(
                        out=h_sb[:, hid : hid + 1],
                        in0=gmask[:, blk : blk + 1],
                        scalar=NEG_BIG,
                        in1=h_sb[:, hid : hid + 1],
                        op0=mybir.AluOpType.mult,
                        op1=mybir.AluOpType.add,
                    )

            # augmented weights Wb = [W | W@a_src | W@a_dst]
            def make_wb(w_in, as_in, ad_in, k, m, tag):
                wb = consts.tile([k, m + 2], F32, tag=f"wb{tag}", name=f"wb{tag}")
                nc.sync.dma_start(out=wb[:, :m], in_=w_in)
                w_sb = sb.tile([k, m], F32, tag="setup_w", name=f"w_sb{tag}")
                nc.sync.dma_start(out=w_sb[:], in_=w_in)
                wT_ps = pp.tile([m, k], F32, tag="xT", name=f"wT_ps{tag}")
                nc.tensor.transpose(out=wT_ps[:], in_=w_sb[:], identity=identity[:k, :k])
                wT_sb = sb.tile([m, k], F32, tag="setup_wT", name=f"wT_sb{tag}")
                nc.vector.tensor_copy(out=wT_sb[:], in_=wT_ps[:])
                av = sb.tile([m, 2], F32, tag="setup_av", name=f"av{tag}")
                nc.sync.dma_start(out=av[:, 0:1], in_=as_in)
                nc.sync.dma_start(out=av[:, 1:2], in_=ad_in)
                v_ps = pp.tile([k, 2], F32, tag="haug", name=f"v_ps{tag}")
                nc.tensor.matmul(
                    out=v_ps[:], lhsT=wT_sb[:], rhs=av[:], start=True, stop=True
                )
                nc.vector.tensor_copy(out=wb[:, m : m + 2], in_=v_ps[:])
                return wb

            wb1 = make_wb(w1_in, a1s_in, a1d_in, in_dim, hid, "1")
            wb2 = make_wb(w2_in, a2s_in, a2d_in, hid, out_dim, "2")

            # bias rows replicated over partitions via K=1 matmul
            ones = consts.tile([1, P], F32)
            nc.vector.memset(ones[:], 1.0)

            def make_brep(b_in, m, tag):
                b_sb = sb.tile([1, m], F32, tag="setup_b", name=f"b_sb{tag}")
                nc.sync.dma_start(out=b_sb[:], in_=b_in)
                b_ps = pp.tile([P, m], F32, tag="haug", name=f"b_ps{tag}")
                nc.tensor.matmul(
                    out=b_ps[:], lhsT=ones[:], rhs=b_sb[:], start=True, stop=True
                )
                brep = consts.tile([P, m], F32, tag=f"brep{tag}", name=f"brep{tag}")
                nc.vector.tensor_copy(out=brep[:], in_=b_ps[:])
                return brep

            b1rep = make_brep(b1_in, hid, "1")
            b2rep = make_brep(b2_in, out_dim, "2")

            # per-node d columns for both layers, filled during the h phases
            d1col = consts.tile([P, nblk], F32)
            d2col = consts.tile([P, nblk], F32)

            # whole slot table resident in SBUF: one DMA, removes the
            # per-block idx-load dependency from the gather critical path
            sumK = int(Kb.sum())
            idx_all = None
            if preload_idx:
                idx_all = consts.tile([P, sumK], I32)
                nc.sync.dma_start(
                    out=idx_all[:],
                    in_=idx_in[:].rearrange("(p s) -> p s", s=sumK),
                )

            # ---- phase 1: h1_aug = x_shard @ Wb1, write local shard ----
            for t in range(nblk):
                x_t = sb.tile([P, in_dim], F32, tag="x_t")
                nc.sync.dma_start(out=x_t[:], in_=xin[t * P : (t + 1) * P, :])
                xT_ps = pp.tile([in_dim, P], F32, tag="xT")
                nc.tensor.transpose(out=xT_ps[:], in_=x_t[:], identity=identity[:])
                xT_sb = sb.tile([in_dim, P], F32, tag="xT_sb")
                nc.vector.tensor_copy(out=xT_sb[:], in_=xT_ps[:])
                h_ps = pp.tile([P, augc], F32, tag="haug")
                nc.tensor.matmul(
                    out=h_ps[:], lhsT=xT_sb[:], rhs=wb1[:], start=True, stop=True
                )
                h_sb = sb.tile([P, augc], F32, tag="h_sb")
                nc.scalar.copy(out=h_sb[:], in_=h_ps[:])
                ghost_fix(h_sb, t)
                nc.vector.tensor_copy(
                    out=d1col[:, t : t + 1], in_=h_sb[:, hid + 1 : hid + 2]
                )
                nc.sync.dma_start(
                    out=sh1[t * P : (t + 1) * P, :augc], in_=h_sb[:]
                )

            # ---- AllGather layer-1 table ------------------------------
            if n_cores > 1:
                nc.gpsimd.collective_compute(
                    "AllGather",
                    mybir.AluOpType.bypass,
                    replica_groups=groups,
                    ins=[sh1[:, :]],
                    outs=[hg1[:, :]],
                )
            else:
                nc.sync.dma_start(out=hg1[:, :], in_=sh1[:, :])

            # ---- per-layer aggregation --------------------------------
            def aggregate(hg, dcol):
                """Yield per-block (num, rz, blk) after the weighted sum.

                num = sum_slots w_e * h[src_e] ([P, hid]); rz = 1/z. The
                caller scales, adds bias, and applies the nonlinearity.
                """
                off = 0
                for b in range(nblk):
                    K = int(Kb[b])
                    if preload_idx:
                        idx_sb = idx_all[:, off // P : off // P + K]
                    else:
                        idx_t = gp.tile([P, K], I32, tag="idx", name="idx_t")
                        nc.sync.dma_start(
                            out=idx_t[:],
                            in_=idx_in[:].rearrange("(p s) -> p s", s=sumK)[
                                :, off // P : off // P + K
                            ],
                        )
                        idx_sb = idx_t[:]
                    gth = gp.tile([P, K * ROWE], F32, tag="gth")
                    for k in range(K):
                        nc.gpsimd.indirect_dma_start(
                            out=gth[:, k * ROWE : (k + 1) * ROWE],
                            out_offset=None,
                            in_=hg[:, :],
                            in_offset=bass.IndirectOffsetOnAxis(
                                ap=idx_sb[:, k : k + 1], axis=0
                            ),
                        )
                    h3 = gth[:].rearrange("p (k e) -> p k e", e=ROWE)
                    sl = h3[:, :, :hid]
                    s_view = h3[:, :, hid : hid + 1].squeeze(2)
                    u = sb.tile([P, K], F32, tag="u")
                    nc.vector.tensor_scalar_add(
                        out=u[:], in0=s_view, scalar1=dcol[:, b : b + 1]
                    )
                    v = sb.tile([P, K], F32, tag="v")
                    nc.vector.scalar_tensor_tensor(
                        out=v[:],
                        in0=u[:],
                        scalar=SLOPE,
                        in1=u[:],
                        op0=mybir.AluOpType.mult,
                        op1=mybir.AluOpType.max,
                    )
                    w = sb.tile([P, K], F32, tag="w")
                    z = sb.tile([P, 1], F32, tag="z")
                    nc.scalar.activation(
                        out=w[:],
                        in_=v[:],
                        func=mybir.ActivationFunctionType.Exp,
                        accum_out=z[:],
                    )
                    wbc = w[:].unsqueeze(2).broadcast_to([P, K, hid])
                    nc.vector.tensor_tensor(
                        out=sl, in0=sl, in1=wbc, op=mybir.AluOpType.mult
                    )
                    num = sb.tile([P, hid], F32, tag="num")
                    nc.vector.tensor_reduce(
                        out=num[:],
                        in_=sl.transpose([0, 2, 1]),
                        axis=mybir.AxisListType.X,
                        op=mybir.AluOpType.add,
                    )
                    zc = sb.tile([P, 1], F32, tag="zc")
                    nc.vector.tensor_scalar_max(out=zc[:], in0=z[:], scalar1=1e-30)
                    rz = sb.tile([P, 1], F32, tag="rz")
                    nc.vector.reciprocal(out=rz[:], in_=zc[:])
                    yield num, rz, b
                    off += P * K

            # layer 1 consume: o = relu(num*rz + b1), h2_aug = o @ Wb2
            import contextlib

            def phase2():
                for num, rz, b in aggregate(hg1, d1col):
                    o = sb.tile([P, hid], F32, tag="o1", name="o")
                    nc.vector.scalar_tensor_tensor(
                        out=o[:],
                        in0=num[:],
                        scalar=rz[:],
                        in1=b1rep[:],
                        op0=mybir.AluOpType.mult,
                        op1=mybir.AluOpType.add,
                    )
                    nc.vector.tensor_scalar_max(out=o[:], in0=o[:], scalar1=0.0)
                    oT_ps = pp.tile([hid, P], F32, tag="oT", name="oT_ps")
                    nc.tensor.transpose(
                        out=oT_ps[:], in_=o[:], identity=identity[:]
                    )
                    oT_sb = sb.tile([hid, P], F32, tag="oT_sb", name="oT_sb")
                    nc.vector.tensor_copy(out=oT_sb[:], in_=oT_ps[:])
                    h2_ps = pp.tile([P, augc], F32, tag="haug", name="h2_ps")
                    nc.tensor.matmul(
                        out=h2_ps[:], lhsT=oT_sb[:], rhs=wb2[:], start=True, stop=True
                    )
                    h2_sb = sb.tile([P, augc], F32, tag="h_sb", name="h2_sb")
                    nc.scalar.copy(out=h2_sb[:], in_=h2_ps[:])
                    ghost_fix(h2_sb, b)
                    nc.vector.tensor_copy(
                        out=d2col[:, b : b + 1], in_=h2_sb[:, hid + 1 : hid + 2]
                    )
                    nc.sync.dma_start(
                        out=sh2[b * P : (b + 1) * P, :augc], in_=h2_sb[:]
                    )

            if bench_reps:
                with tc.For_i(0, bench_reps, 1):
                    phase2()
            else:
                phase2()

            if n_cores > 1:
                nc.gpsimd.collective_compute(
                    "AllGather",
                    mybir.AluOpType.bypass,
                    replica_groups=groups,
                    ins=[sh2[:, :]],
                    outs=[hg2[:, :]],
                )
            else:
                nc.sync.dma_start(out=hg2[:, :], in_=sh2[:, :])

            # layer 2 consume: o = num*rz + b2, row-L2-normalize, store
            def phase3():
                for num, rz, b in aggregate(hg2, d2col):
                    o = sb.tile([P, out_dim], F32, tag="o2", name="o")
                    nc.vector.scalar_tensor_tensor(
                        out=o[:],
                        in0=num[:],
                        scalar=rz[:],
                        in1=b2rep[:],
                        op0=mybir.AluOpType.mult,
                        op1=mybir.AluOpType.add,
                    )
                    sq = sb.tile([P, out_dim], F32, tag="sq", name="sq")
                    ss = sb.tile([P, 1], F32, tag="ss", name="ss")
                    nc.vector.tensor_tensor(
                        out=sq[:], in0=o[:], in1=o[:], op=mybir.AluOpType.mult
                    )
                    nc.vector.tensor_reduce(
                        out=ss[:],
                        in_=sq[:],
                        axis=mybir.AxisListType.X,
                        op=mybir.AluOpType.add,
                    )
                    nc.vector.tensor_scalar_max(
                        out=ss[:], in0=ss[:], scalar1=1e-20
                    )
                    lns = sb.tile([P, 1], F32, tag="lns", name="lns")
                    nc.scalar.activation(
                        out=lns[:], in_=ss[:], func=mybir.ActivationFunctionType.Ln
                    )
                    rn = sb.tile([P, 1], F32, tag="rn", name="rn")
                    nc.scalar.activation(
                        out=rn[:],
                        in_=lns[:],
                        func=mybir.ActivationFunctionType.Exp,
                        scale=-0.5,
                    )
                    of = sb.tile([P, out_dim], F32, tag="of", name="of")
                    nc.vector.tensor_scalar_mul(out=of[:], in0=o[:], scalar1=rn[:])
                    nc.sync.dma_start(
                        out=out_t[b * P : (b + 1) * P, :], in_=of[:]
                    )

            if bench_reps:
                with tc.For_i(0, bench_reps, 1):
                    phase3()
            else:
                phase3()

    nc.compile()
    return nc


# --------------------------------------------------------------------------
# Entry point
# --------------------------------------------------------------------------

def kernel(
    x,
    edge_index,
    W1,
    att_src1,
    att_dst1,
    b1,
    W2,
    att_src2,
    att_dst2,
    b2,
    _n_cores=8,
    _slot_budget=64,
    _trace=False,
):
    global LAST_RESULTS, LAST_META
    x = np.asarray(x, np.float32)
    edge_index = np.asarray(edge_index)
    src = edge_index[0].astype(np.int64)
    dst = edge_index[1].astype(np.int64)
    n_nodes = x.shape[0]
    in_dim = x.shape[1]
    hid = np.asarray(W1).shape[1]
    out_dim = np.asarray(W2).shape[1]

    key = (
        hash(edge_index.tobytes()),
        n_nodes,
        in_dim,
        hid,
        out_dim,
        _n_cores,
        _slot_budget,
    )
    if key in _PROGRAM_CACHE:
        meta, nc = _PROGRAM_CACHE[key]
    else:
        meta = _host_prep(src, dst, n_nodes, _n_cores, _slot_budget)
        nc = _build_nc(meta, _n_cores, in_dim, hid, out_dim)
        _PROGRAM_CACHE[key] = (meta, nc)
    LAST_META = meta
    shard = meta["shard"]

    nblk = meta["nblk"]
    gmask = (
        np.arange(meta["shard"]).reshape(nblk, P).T >= meta["shard_orig"]
    ).astype(np.float32)
    common = {
        "ident": np.eye(P, dtype=np.float32),
        "gmask": np.ascontiguousarray(gmask),
        "W1": np.ascontiguousarray(W1, np.float32),
        "a1s": np.ascontiguousarray(np.asarray(att_src1, np.float32).reshape(hid, 1)),
        "a1d": np.ascontiguousarray(np.asarray(att_dst1, np.float32).reshape(hid, 1)),
        "b1": np.ascontiguousarray(np.asarray(b1, np.float32).reshape(1, hid)),
        "W2": np.ascontiguousarray(W2, np.float32),
        "a2s": np.ascontiguousarray(np.asarray(att_src2, np.float32).reshape(out_dim, 1)),
        "a2d": np.ascontiguousarray(np.asarray(att_dst2, np.float32).reshape(out_dim, 1)),
        "b2": np.ascontiguousarray(np.asarray(b2, np.float32).reshape(1, out_dim)),
    }
    in_maps = []
    for c in range(_n_cores):
        ids = meta["perm_new2old"][c * shard : (c + 1) * shard]
        x_shard = np.zeros((shard, in_dim), np.float32)
        real = ids >= 0
        x_shard[real] = x[ids[real]]
        in_maps.append(dict(common, x_shard=x_shard, idx=meta["idx_flat"][c]))

    res = bass_utils.run_bass_kernel_spmd(
        nc, in_maps, core_ids=list(range(_n_cores)), trace=_trace
    )
    LAST_RESULTS = res

    full = np.empty((n_nodes, out_dim), np.float32)
    for c in range(_n_cores):
        ids = meta["perm_new2old"][c * shard : (c + 1) * shard]
        real = ids >= 0
        full[ids[real]] = res.results[c]["out"][real]
    return full



# revision 15
# speedup vs baseline: 402.5002x; 402.5002x over previous
"""Two-layer GAT (single-head, PyG-style) on 8 Trainium2 NeuronCores — v2.

Strategy (destination-sharded, as v1) with a rebuilt gather pipeline:
  - Table rows are 256B bf16 (h[64] bf16 | s f32 in 2 bf16 slots | pad)
    instead of 512B f32: random-access HBM reads are ~3.4x faster per row
    at 256B, and bf16 h is well within the 2e-2 tolerance (s stays f32).
  - Per-edge rows are fetched with batched SWDGE dma_gather (<=1024 rows
    per instruction, round-robined over 4 SWDGE queues) instead of one
    qPoolDynamic indirect DMA per slot column: ~10x fewer Pool-engine
    instructions and ~3x more DMA-queue parallelism.
  - dma_gather indices are int16 (<=32767) but the table has 50176 rows:
    slots are split by *source-id parity* and fetched from even/odd
    strided views of the table (elem_step=512B, index = row >> 1, max
    25088). A host-side greedy discrepancy pass chooses which nodes get
    even/odd ids (within each 128-node block, 64/64) so that each
    destination's in-edges split ~evenly and per-block slot counts stay
    near ceil(K/2) per parity.
  - The table AllGather moves bf16 rows (half the bytes of v1), and
    padding slots cycle over every core's ghost rows: a single guard row
    would serialize ~100k same-address HBM reads on one bank (measured
    ~6x slowdown of the whole aggregation phase).
"""

import numpy as np

import concourse.bacc as bacc
import concourse.bass as bass
import concourse.mybir as mybir
import concourse.tile as tile
from concourse import bass_utils

F32 = mybir.dt.float32
BF16 = mybir.dt.bfloat16
I16 = mybir.dt.int16
P = 128
ROWE = 128          # table row = 128 bf16 = 256B
SLOPE = 0.2
NEG_BIG = -1.0e9
NG = 1024           # max rows per dma_gather (SWDGE ring limit)

LAST_RESULTS = None
LAST_META = None

_PROGRAM_CACHE = {}


# --------------------------------------------------------------------------
# Host-side preprocessing
# --------------------------------------------------------------------------

def _parity_assign(src, dst, n_nodes, n_cores, shard_orig):
    """Greedy discrepancy: pick ~half of each core's nodes for even ids so
    each destination's in-edges split evenly between even and odd sources.

    Budget: per core at most ceil((shard_orig+pad)/2) per class (block
    re-binning later needs 64/64 per block, ghosts absorb the remainder).
    Returns sigma[old_id] in {0 (even), 1}.
    """
    o = np.argsort(src, kind="stable")
    odst = dst[o]
    starts = np.searchsorted(src[o], np.arange(n_nodes + 1))
    imb = np.zeros(n_nodes, np.int64)
    sigma = np.full(n_nodes, -1, np.int8)
    deg = np.bincount(dst, minlength=n_nodes)
    nblk = -(-(shard_orig + 1) // P)
    half = (nblk * P) // 2 - 1   # leave >=1 ghost slot per parity (guards)
    counts = np.zeros((n_cores, 2), np.int64)
    # pass 0: quadratic greedy (sigma = -sign(sum of dst imbalances))
    for c in range(n_cores):
        ids = np.arange(c * shard_orig, (c + 1) * shard_orig)
        order = np.argsort(-deg[ids], kind="stable")
        budget = [half, half]
        for node in ids[order]:
            dd = odst[starts[node]:starts[node + 1]]
            t = int(imb[dd].sum()) if len(dd) else 0
            s = 0 if (t <= 0) else 1
            if budget[s] == 0:
                s = 1 - s
            sigma[node] = s
            budget[s] -= 1
            if len(dd):
                np.add.at(imb, dd, 1 - 2 * s)
        counts[c, 0] = half - budget[0]
        counts[c, 1] = half - budget[1]
    # improvement passes: flip a node when it lowers sum(I^2) and budgets
    # stay legal. delta(flip) = sum over dsts of ((I -+ 2)^2 - I^2).
    for _ in range(3):
        flips = 0
        for node in range(n_nodes):
            c = node // shard_orig
            s = int(sigma[node])
            if counts[c, 1 - s] >= half:
                continue
            dd = odst[starts[node]:starts[node + 1]]
            if not len(dd):
                continue
            sgn = 1 - 2 * s          # current contribution per edge
            # flipping changes each dst's I by -2*sgn
            delta = int(-4 * sgn * imb[dd].sum() + 4 * len(dd))
            if delta < 0:
                sigma[node] = 1 - s
                counts[c, s] -= 1
                counts[c, 1 - s] += 1
                np.add.at(imb, dd, -2 * sgn)
                flips += 1
        if not flips:
            break
    return sigma


def _host_prep(src, dst, n_nodes, n_cores):
    assert n_nodes % n_cores == 0
    shard_orig = n_nodes // n_cores
    nblk = -(-(shard_orig + 1) // P)
    shard = nblk * P
    np_tot = n_cores * shard

    sigma = _parity_assign(src, dst, n_nodes, n_cores, shard_orig)

    # per-destination parity in-degrees (parity of a source = sigma, fixed
    # regardless of which block/slot it ends up in)
    deg_e = np.zeros(n_nodes, np.int64)
    deg_o = np.zeros(n_nodes, np.int64)
    np.add.at(deg_e, dst[sigma[src] == 0], 1)
    np.add.at(deg_o, dst[sigma[src] == 1], 1)

    # re-bin: per core, sort each class by max(deg_e, deg_o) desc and fill
    # blocks with 64 evens + 64 odds (ghosts absorb exhausted classes)
    perm_new2old = np.full(np_tot, -1, np.int64)
    perm_old2new = np.empty(n_nodes, np.int64)
    key = np.maximum(deg_e, deg_o)
    for c in range(n_cores):
        ids = np.arange(c * shard_orig, (c + 1) * shard_orig)
        ev = ids[sigma[ids] == 0]
        od = ids[sigma[ids] == 1]
        ev = ev[np.argsort(-key[ev], kind="stable")]
        od = od[np.argsort(-key[od], kind="stable")]
        ei = oi = 0
        for b in range(nblk):
            ne = min(P // 2, len(ev) - ei)
            no = min(P // 2, len(od) - oi)
            base = c * shard + b * P
            perm_new2old[base + 0:base + 2 * ne:2] = ev[ei:ei + ne]
            perm_new2old[base + 1:base + 2 * no + 1:2] = od[oi:oi + no]
            ei += ne
            oi += no
        sl = perm_new2old[c * shard:(c + 1) * shard]
        ok = sl >= 0
        perm_old2new[sl[ok]] = c * shard + np.arange(shard)[ok]

    new_src = perm_old2new[src]
    new_dst = perm_old2new[dst]

    # guard rows: every core's ghost slots work (their shard writes set
    # s=-1e9). Padding cycles over all of them — a single guard row would
    # be a same-bank HBM hotspot under 8-core random-read load.
    ghost = np.where(perm_new2old < 0)[0]
    guards = (ghost[ghost % 2 == 0], ghost[ghost % 2 == 1])
    assert len(guards[0]) and len(guards[1])

    Kpar = np.zeros((2, nblk), np.int64)   # [parity, block] max col count
    deg_par = np.zeros((2, np_tot), np.int64)
    par = (new_src & 1).astype(np.int64)
    np.add.at(deg_par, (par, new_dst), 1)
    for pbit in (0, 1):
        dp = deg_par[pbit].reshape(n_cores, nblk, P)
        Kpar[pbit] = np.maximum(dp.max(axis=(0, 2)), 1)

    # slot tables [np_tot, Kmax] per parity, padding spread over all guards
    tables = []
    for pbit in (0, 1):
        kmax = int(Kpar[pbit].max())
        g = guards[pbit]
        fill = np.arange(np_tot * kmax).reshape(np_tot, kmax)
        tab = g[fill % len(g)]
        sel = par == pbit
        sdst = new_dst[sel]
        ssrc = new_src[sel]
        o = np.argsort(sdst, kind="stable")
        sdst = sdst[o]
        ssrc = ssrc[o]
        counts = np.bincount(sdst, minlength=np_tot)
        st = np.concatenate([[0], np.cumsum(counts)[:-1]])
        slot = np.arange(len(sdst)) - st[sdst]
        tab[sdst, slot] = ssrc
        tables.append(tab)

    # idx16 images per core: per block, regions [E cols | O cols],
    # positions k-major (pos = k*128 + p), idx value = new_src >> 1.
    # image[pos % 16, pos // 16] wrapped; replicated to 128 partitions.
    sumK = int(Kpar.sum())
    idx_imgs = np.empty((n_cores, P, sumK * 8), np.int16)
    for c in range(n_cores):
        cols = []
        for b in range(nblk):
            for pbit in (0, 1):
                K = int(Kpar[pbit][b])
                blk = tables[pbit][c * shard + b * P:c * shard + (b + 1) * P, :K]
                pos_val = (blk.T >> 1).reshape(-1)      # k-major
                cols.append(pos_val.reshape(-1, 16).T)  # [16, K*8]
        img16 = np.concatenate(cols, axis=1).astype(np.int16)
        img = np.tile(img16, (8, 1))
        idx_imgs[c] = img

    return dict(
        perm_new2old=perm_new2old,
        perm_old2new=perm_old2new,
        shard=shard,
        shard_orig=shard_orig,
        nblk=nblk,
        np_tot=np_tot,
        Kpar=Kpar,
        sumK=sumK,
        idx_imgs=idx_imgs,
    )


# --------------------------------------------------------------------------
# Device program
# --------------------------------------------------------------------------

def _build_nc(meta, n_cores, in_dim, hid, out_dim, bench_reps=0,
              bench_gather_only=False):
    shard = meta["shard"]
    nblk = meta["nblk"]
    np_tot = meta["np_tot"]
    Kpar = meta["Kpar"]
    shard_orig = meta["shard_orig"]
    sumK = meta["sumK"]
    augc = hid + 2  # h | s | d

    nc = bacc.Bacc(
        "TRN2", target_bir_lowering=False, debug=False,
        num_devices=n_cores, num_swdge_queues=4,
    )

    xin = nc.dram_tensor("x_shard", [shard, in_dim], F32, kind="ExternalInput").ap()
    idx_in = nc.dram_tensor("idx", [P, sumK * 8], I16, kind="ExternalInput").ap()
    gmask_in = nc.dram_tensor("gmask", [P, nblk], F32, kind="ExternalInput").ap()
    ident_in = nc.dram_tensor("ident", [P, P], F32, kind="ExternalInput").ap()
    w1_in = nc.dram_tensor("W1", [in_dim, hid], F32, kind="ExternalInput").ap()
    a1s_in = nc.dram_tensor("a1s", [hid, 1], F32, kind="ExternalInput").ap()
    a1d_in = nc.dram_tensor("a1d", [hid, 1], F32, kind="ExternalInput").ap()
    b1_in = nc.dram_tensor("b1", [1, hid], F32, kind="ExternalInput").ap()
    w2_in = nc.dram_tensor("W2", [hid, out_dim], F32, kind="ExternalInput").ap()
    a2s_in = nc.dram_tensor("a2s", [out_dim, 1], F32, kind="ExternalInput").ap()
    a2d_in = nc.dram_tensor("a2d", [out_dim, 1], F32, kind="ExternalInput").ap()
    b2_in = nc.dram_tensor("b2", [1, out_dim], F32, kind="ExternalInput").ap()
    out_t = nc.dram_tensor("out", [shard, out_dim], F32, kind="ExternalOutput").ap()

    groups = [list(range(n_cores))]
    qctr = [0]

    def next_q():
        q = qctr[0] & 3
        qctr[0] += 1
        return q

    with tile.TileContext(nc) as tc:
        with (
            tc.tile_pool(name="dram", bufs=1, space="DRAM") as dram,
            tc.tile_pool(name="consts", bufs=1) as consts,
            tc.tile_pool(name="sb", bufs=3) as sb,
            tc.tile_pool(name="gather", bufs=6) as gp,
            tc.tile_pool(name="prodp", bufs=5) as prp,
            tc.tile_pool(name="psum", bufs=2, space="PSUM") as pp,
        ):
            shared = "Shared" if n_cores > 1 else "Local"
            hg1 = dram.tile([np_tot, ROWE], BF16, addr_space=shared)
            hg2 = dram.tile([np_tot, ROWE], BF16, addr_space=shared)

            identity = consts.tile([P, P], F32)
            nc.sync.dma_start(out=identity[:], in_=ident_in)
            gmask = consts.tile([P, nblk], F32)
            nc.sync.dma_start(out=gmask[:], in_=gmask_in)

            idx_all = consts.tile([P, sumK * 8], I16)
            nc.sync.dma_start(out=idx_all[:], in_=idx_in)

            def ghost_fix(h_sb, blk):
                if (blk + 1) * P > shard_orig:
                    nc.vector.scalar_tensor_tensor(
                        out=h_sb[:, hid:hid + 1],
                        in0=gmask[:, blk:blk + 1],
                        scalar=NEG_BIG,
                        in1=h_sb[:, hid:hid + 1],
                        op0=mybir.AluOpType.mult,
                        op1=mybir.AluOpType.add,
                    )

            def make_wb(w_in, as_in, ad_in, k, m, tag):
                wb = consts.tile([k, m + 2], F32, tag=f"wb{tag}", name=f"wb{tag}")
                nc.sync.dma_start(out=wb[:, :m], in_=w_in)
                w_sb = sb.tile([k, m], F32, tag="setup_w", name=f"w_sb{tag}")
                nc.sync.dma_start(out=w_sb[:], in_=w_in)
                wT_ps = pp.tile([m, k], F32, tag="xT", name=f"wT_ps{tag}")
                nc.tensor.transpose(out=wT_ps[:], in_=w_sb[:], identity=identity[:k, :k])
                wT_sb = sb.tile([m, k], F32, tag="setup_wT", name=f"wT_sb{tag}")
                nc.vector.tensor_copy(out=wT_sb[:], in_=wT_ps[:])
                av = sb.tile([m, 2], F32, tag="setup_av", name=f"av{tag}")
                nc.sync.dma_start(out=av[:, 0:1], in_=as_in)
                nc.sync.dma_start(out=av[:, 1:2], in_=ad_in)
                v_ps = pp.tile([k, 2], F32, tag="haug", name=f"v_ps{tag}")
                nc.tensor.matmul(
                    out=v_ps[:], lhsT=wT_sb[:], rhs=av[:], start=True, stop=True
                )
                nc.vector.tensor_copy(out=wb[:, m:m + 2], in_=v_ps[:])
                return wb

            wb1 = make_wb(w1_in, a1s_in, a1d_in, in_dim, hid, "1")
            wb2 = make_wb(w2_in, a2s_in, a2d_in, hid, out_dim, "2")

            ones = consts.tile([1, P], F32)
            nc.vector.memset(ones[:], 1.0)

            def make_brep(b_in, m, tag):
                b_sb = sb.tile([1, m], F32, tag="setup_b", name=f"b_sb{tag}")
                nc.sync.dma_start(out=b_sb[:], in_=b_in)
                b_ps = pp.tile([P, m], F32, tag="haug", name=f"b_ps{tag}")
                nc.tensor.matmul(
                    out=b_ps[:], lhsT=ones[:], rhs=b_sb[:], start=True, stop=True
                )
                brep = consts.tile([P, m], F32, tag=f"brep{tag}", name=f"brep{tag}")
                nc.vector.tensor_copy(out=brep[:], in_=b_ps[:])
                return brep

            b1rep = make_brep(b1_in, hid, "1")
            b2rep = make_brep(b2_in, out_dim, "2")

            d1col = consts.tile([P, nblk], F32)
            d2col = consts.tile([P, nblk], F32)

            sh1 = dram.tile([shard, ROWE], BF16)
            sh2 = dram.tile([shard, ROWE], BF16)

            def to_row(h_sb, blk, dcol, sh):
                """Convert haug f32 [P, augc] -> bf16 row and store to sh."""
                nc.vector.tensor_copy(
                    out=dcol[:, blk:blk + 1], in_=h_sb[:, hid + 1:hid + 2]
                )
                ghost_fix(h_sb, blk)
                hrow = sb.tile([P, ROWE], BF16, tag="hrow", name="hrow")
                nc.vector.tensor_copy(out=hrow[:, :hid], in_=h_sb[:, :hid])
                nc.vector.tensor_copy(
                    out=hrow[:, hid:hid + 2].bitcast(F32),
                    in_=h_sb[:, hid:hid + 1],
                )
                nc.sync.dma_start(
                    out=sh[blk * P:(blk + 1) * P, :], in_=hrow[:]
                )

            # ---- phase 1: h1_aug = x_shard @ Wb1 ----------------------
            for t in range(nblk):
                x_t = sb.tile([P, in_dim], F32, tag="x_t")
                nc.sync.dma_start(out=x_t[:], in_=xin[t * P:(t + 1) * P, :])
                xT_ps = pp.tile([in_dim, P], F32, tag="xT")
                nc.tensor.transpose(out=xT_ps[:], in_=x_t[:], identity=identity[:])
                xT_sb = sb.tile([in_dim, P], F32, tag="xT_sb")
                nc.vector.tensor_copy(out=xT_sb[:], in_=xT_ps[:])
                h_ps = pp.tile([P, augc], F32, tag="haug")
                nc.tensor.matmul(
                    out=h_ps[:], lhsT=xT_sb[:], rhs=wb1[:], start=True, stop=True
                )
                h_sb = sb.tile([P, augc], F32, tag="h_sb")
                nc.scalar.copy(out=h_sb[:], in_=h_ps[:])
                to_row(h_sb, t, d1col, sh1)

            if n_cores > 1:
                nc.gpsimd.collective_compute(
                    "AllGather", mybir.AluOpType.bypass,
                    replica_groups=groups, ins=[sh1[:, :]], outs=[hg1[:, :]],
                )
            else:
                nc.sync.dma_start(out=hg1[:, :], in_=sh1[:, :])

            # ---- aggregation ------------------------------------------
            def aggregate(hg, dcol):
                hgv = hg[:].rearrange("(a b) e -> a (b e)", b=2)
                views = (hgv[:, :ROWE], hgv[:, ROWE:])
                icol = 0
                for b in range(nblk):
                    Ke = int(Kpar[0][b])
                    Ko = int(Kpar[1][b])
                    K = Ke + Ko
                    gth = gp.tile([P, K * ROWE], BF16, tag="gth")
                    co = 0
                    for pbit, Kp in ((0, Ke), (1, Ko)):
                        done = 0
                        while done < Kp:
                            nk = min(Kp - done, NG // P)
                            nidx = nk * P
                            nc.gpsimd.dma_gather(
                                out_ap=gth[:, (co + done) * ROWE:
                                           (co + done + nk) * ROWE]
                                    .rearrange("p (j e) -> p j e", e=ROWE),
                                in_ap=views[pbit],
                                idxs_ap=idx_all[:, icol:icol + nidx // 16],
                                num_idxs=nidx,
                                num_idxs_reg=nidx,
                                elem_size=ROWE,
                                elem_step=2 * ROWE,
                                queue_num=next_q(),
                            )
                            icol += nidx // 16
                            done += nk
                        co += Kp
                    if bench_gather_only:
                        dmy = sb.tile([P, 1], BF16, tag="dmy", name="dmy")
                        nc.vector.tensor_copy(out=dmy[:], in_=gth[:, :1])
                        yield None, None, b
                        continue
                    g3 = gth[:].rearrange("p (k e) -> p k e", e=ROWE)
                    sview = g3[:, :, hid:hid + 2].bitcast(F32).squeeze(2)
                    u = sb.tile([P, K], F32, tag="u")
                    nc.vector.tensor_scalar_add(
                        out=u[:], in0=sview, scalar1=dcol[:, b:b + 1]
                    )
                    v = sb.tile([P, K], F32, tag="v")
                    nc.vector.scalar_tensor_tensor(
                        out=v[:], in0=u[:], scalar=SLOPE, in1=u[:],
                        op0=mybir.AluOpType.mult, op1=mybir.AluOpType.max,
                    )
                    w = sb.tile([P, K], F32, tag="w")
                    z = sb.tile([P, 1], F32, tag="z")
                    nc.scalar.activation(
                        out=w[:], in_=v[:],
                        func=mybir.ActivationFunctionType.Exp, accum_out=z[:],
                    )
                    wbc = w[:].unsqueeze(2).broadcast_to([P, K, hid])
                    prod = prp.tile([P, K * hid], F32, tag="prod")
                    p3 = prod[:].rearrange("p (k e) -> p k e", e=hid)
                    nc.vector.tensor_tensor(
                        out=p3, in0=g3[:, :, :hid], in1=wbc,
                        op=mybir.AluOpType.mult,
                    )
                    num = sb.tile([P, hid], F32, tag="num")
                    nc.vector.tensor_reduce(
                        out=num[:], in_=p3.transpose([0, 2, 1]),
                        axis=mybir.AxisListType.X, op=mybir.AluOpType.add,
                    )
                    zc = sb.tile([P, 1], F32, tag="zc")
                    nc.vector.tensor_scalar_max(out=zc[:], in0=z[:], scalar1=1e-30)
                    rz = sb.tile([P, 1], F32, tag="rz")
                    nc.vector.reciprocal(out=rz[:], in_=zc[:])
                    yield num, rz, b

            # ---- phase 2: aggregate layer 1, compute h2_aug -----------
            def phase2():
                for num, rz, b in aggregate(hg1, d1col):
                    if bench_gather_only:
                        hrow = sb.tile([P, ROWE], BF16, tag="hrow", name="hrow")
                        nc.vector.memset(hrow[:], 0.0)
                        nc.sync.dma_start(
                            out=sh2[b * P:(b + 1) * P, :], in_=hrow[:])
                        continue
                    o = sb.tile([P, hid], F32, tag="o1", name="o")
                    nc.vector.scalar_tensor_tensor(
                        out=o[:], in0=num[:], scalar=rz[:], in1=b1rep[:],
                        op0=mybir.AluOpType.mult, op1=mybir.AluOpType.add,
                    )
                    nc.vector.tensor_scalar_max(out=o[:], in0=o[:], scalar1=0.0)
                    oT_ps = pp.tile([hid, P], F32, tag="oT", name="oT_ps")
                    nc.tensor.transpose(out=oT_ps[:], in_=o[:], identity=identity[:])
                    oT_sb = sb.tile([hid, P], F32, tag="oT_sb", name="oT_sb")
                    nc.vector.tensor_copy(out=oT_sb[:], in_=oT_ps[:])
                    h2_ps = pp.tile([P, augc], F32, tag="haug", name="h2_ps")
                    nc.tensor.matmul(
                        out=h2_ps[:], lhsT=oT_sb[:], rhs=wb2[:],
                        start=True, stop=True,
                    )
                    h2_sb = sb.tile([P, augc], F32, tag="h_sb", name="h2_sb")
                    nc.scalar.copy(out=h2_sb[:], in_=h2_ps[:])
                    to_row(h2_sb, b, d2col, sh2)

            if bench_reps:
                with tc.For_i(0, bench_reps, 1):
                    phase2()
            else:
                phase2()

            if n_cores > 1:
                nc.gpsimd.collective_compute(
                    "AllGather", mybir.AluOpType.bypass,
                    replica_groups=groups, ins=[sh2[:, :]], outs=[hg2[:, :]],
                )
            else:
                nc.sync.dma_start(out=hg2[:, :], in_=sh2[:, :])

            # ---- phase 3: aggregate layer 2, normalize, store ---------
            def phase3():
                for num, rz, b in aggregate(hg2, d2col):
                    if bench_gather_only:
                        of = sb.tile([P, out_dim], F32, tag="of", name="of")
                        nc.vector.memset(of[:], 0.0)
                        nc.sync.dma_start(
                            out=out_t[b * P:(b + 1) * P, :], in_=of[:])
                        continue
                    o = sb.tile([P, out_dim], F32, tag="o2", name="o")
                    nc.vector.scalar_tensor_tensor(
                        out=o[:], in0=num[:], scalar=rz[:], in1=b2rep[:],
                        op0=mybir.AluOpType.mult, op1=mybir.AluOpType.add,
                    )
                    sq = sb.tile([P, out_dim], F32, tag="sq", name="sq")
                    ss = sb.tile([P, 1], F32, tag="ss", name="ss")
                    nc.vector.tensor_tensor(
                        out=sq[:], in0=o[:], in1=o[:], op=mybir.AluOpType.mult
                    )
                    nc.vector.tensor_reduce(
                        out=ss[:], in_=sq[:], axis=mybir.AxisListType.X,
                        op=mybir.AluOpType.add,
                    )
                    nc.vector.tensor_scalar_max(out=ss[:], in0=ss[:], scalar1=1e-20)
                    lns = sb.tile([P, 1], F32, tag="lns", name="lns")
                    nc.scalar.activation(
                        out=lns[:], in_=ss[:], func=mybir.ActivationFunctionType.Ln
                    )
                    rn = sb.tile([P, 1], F32, tag="rn", name="rn")
                    nc.scalar.activation(
                        out=rn[:], in_=lns[:],
                        func=mybir.ActivationFunctionType.Exp, scale=-0.5,
                    )
                    of = sb.tile([P, out_dim], F32, tag="of", name="of")
                    nc.vector.tensor_scalar_mul(out=of[:], in0=o[:], scalar1=rn[:])
                    nc.sync.dma_start(out=out_t[b * P:(b + 1) * P, :], in_=of[:])

            if bench_reps:
                with tc.For_i(0, bench_reps, 1):
                    phase3()
            else:
                phase3()

    nc.compile()
    return nc


# --------------------------------------------------------------------------
# Entry point
# --------------------------------------------------------------------------

def kernel(
    x, edge_index, W1, att_src1, att_dst1, b1, W2, att_src2, att_dst2, b2,
    _n_cores=8,
):
    global LAST_RESULTS, LAST_META
    x = np.asarray(x, np.float32)
    edge_index = np.asarray(edge_index)
    src = edge_index[0].astype(np.int64)
    dst = edge_index[1].astype(np.int64)
    n_nodes = x.shape[0]
    in_dim = x.shape[1]
    hid = np.asarray(W1).shape[1]
    out_dim = np.asarray(W2).shape[1]

    key = (hash(edge_index.tobytes()), n_nodes, in_dim, hid, out_dim, _n_cores)
    if key in _PROGRAM_CACHE:
        meta, nc = _PROGRAM_CACHE[key]
    else:
        meta = _host_prep(src, dst, n_nodes, _n_cores)
        nc = _build_nc(meta, _n_cores, in_dim, hid, out_dim)
        _PROGRAM_CACHE[key] = (meta, nc)
    LAST_META = meta
    shard = meta["shard"]
    nblk = meta["nblk"]

    common = {
        "ident": np.eye(P, dtype=np.float32),
        "W1": np.ascontiguousarray(W1, np.float32),
        "a1s": np.ascontiguousarray(np.asarray(att_src1, np.float32).reshape(hid, 1)),
        "a1d": np.ascontiguousarray(np.asarray(att_dst1, np.float32).reshape(hid, 1)),
        "b1": np.ascontiguousarray(np.asarray(b1, np.float32).reshape(1, hid)),
        "W2": np.ascontiguousarray(W2, np.float32),
        "a2s": np.ascontiguousarray(np.asarray(att_src2, np.float32).reshape(out_dim, 1)),
        "a2d": np.ascontiguousarray(np.asarray(att_dst2, np.float32).reshape(out_dim, 1)),
        "b2": np.ascontiguousarray(np.asarray(b2, np.float32).reshape(1, out_dim)),
    }
    in_maps = []
    for c in range(_n_cores):
        ids = meta["perm_new2old"][c * shard:(c + 1) * shard]
        x_shard = np.zeros((shard, in_dim), np.float32)
        real = ids >= 0
        x_shard[real] = x[ids[real]]
        gmask = (ids < 0).reshape(nblk, P).T.astype(np.float32)
        in_maps.append(dict(
            common, x_shard=x_shard, idx=meta["idx_imgs"][c],
            gmask=np.ascontiguousarray(gmask),
        ))

    res = bass_utils.run_bass_kernel_spmd(
        nc, in_maps, core_ids=list(range(_n_cores))
    )
    LAST_RESULTS = res

    full = np.empty((n_nodes, out_dim), np.float32)
    for c in range(_n_cores):
        ids = meta["perm_new2old"][c * shard:(c + 1) * shard]
        real = ids >= 0
        full[ids[real]] = res.results[c]["out"][real]
    return full


# revision 16
# speedup vs baseline: 405.9076x; 1.0085x over previous
"""Two-layer GAT (single-head, PyG-style) on 8 Trainium2 NeuronCores — v2.

Strategy (destination-sharded, as v1) with a rebuilt gather pipeline:
  - Table rows are 256B bf16 (h[64] bf16 | s f32 in 2 bf16 slots | pad)
    instead of 512B f32: random-access HBM reads are ~3.4x faster per row
    at 256B, and bf16 h is well within the 2e-2 tolerance (s stays f32).
  - Per-edge rows are fetched with batched SWDGE dma_gather (<=1024 rows
    per instruction, round-robined over 4 SWDGE queues) instead of one
    qPoolDynamic indirect DMA per slot column: ~10x fewer Pool-engine
    instructions and ~3x more DMA-queue parallelism.
  - dma_gather indices are int16 (<=32767) but the table has 50176 rows:
    slots are split by *source-id parity* and fetched from even/odd
    strided views of the table (elem_step=512B, index = row >> 1, max
    25088). A host-side greedy discrepancy pass chooses which nodes get
    even/odd ids (within each 128-node block, 64/64) so that each
    destination's in-edges split ~evenly and per-block slot counts stay
    near ceil(K/2) per parity.
  - The table AllGather moves bf16 rows (half the bytes of v1), and
    padding slots cycle over every core's ghost rows: a single guard row
    would serialize ~100k same-address HBM reads on one bank (measured
    ~6x slowdown of the whole aggregation phase).
"""

import numpy as np

import concourse.bacc as bacc
import concourse.bass as bass
import concourse.mybir as mybir
import concourse.tile as tile
from concourse import bass_utils

F32 = mybir.dt.float32
BF16 = mybir.dt.bfloat16
I16 = mybir.dt.int16
P = 128
ROWE = 128          # table row = 128 bf16 = 256B
SLOPE = 0.2
NEG_BIG = -1.0e9
NG = 1024           # max rows per dma_gather (SWDGE ring limit)

LAST_RESULTS = None
LAST_META = None

_PROGRAM_CACHE = {}


# --------------------------------------------------------------------------
# Host-side preprocessing
# --------------------------------------------------------------------------

def _parity_assign(src, dst, n_nodes, n_cores, shard_orig):
    """Greedy discrepancy: pick ~half of each core's nodes for even ids so
    each destination's in-edges split evenly between even and odd sources.

    Budget: per core at most ceil((shard_orig+pad)/2) per class (block
    re-binning later needs 64/64 per block, ghosts absorb the remainder).
    Returns sigma[old_id] in {0 (even), 1}.
    """
    o = np.argsort(src, kind="stable")
    odst = dst[o]
    starts = np.searchsorted(src[o], np.arange(n_nodes + 1))
    imb = np.zeros(n_nodes, np.int64)
    sigma = np.full(n_nodes, -1, np.int8)
    deg = np.bincount(dst, minlength=n_nodes)
    nblk = -(-(shard_orig + 1) // P)
    half = (nblk * P) // 2 - 1   # leave >=1 ghost slot per parity (guards)
    counts = np.zeros((n_cores, 2), np.int64)
    # pass 0: quadratic greedy (sigma = -sign(sum of dst imbalances))
    for c in range(n_cores):
        ids = np.arange(c * shard_orig, (c + 1) * shard_orig)
        order = np.argsort(-deg[ids], kind="stable")
        budget = [half, half]
        for node in ids[order]:
            dd = odst[starts[node]:starts[node + 1]]
            t = int(imb[dd].sum()) if len(dd) else 0
            s = 0 if (t <= 0) else 1
            if budget[s] == 0:
                s = 1 - s
            sigma[node] = s
            budget[s] -= 1
            if len(dd):
                np.add.at(imb, dd, 1 - 2 * s)
        counts[c, 0] = half - budget[0]
        counts[c, 1] = half - budget[1]
    # improvement passes: flip a node when it lowers sum(I^2) and budgets
    # stay legal. delta(flip) = sum over dsts of ((I -+ 2)^2 - I^2).
    for _ in range(3):
        flips = 0
        for node in range(n_nodes):
            c = node // shard_orig
            s = int(sigma[node])
            if counts[c, 1 - s] >= half:
                continue
            dd = odst[starts[node]:starts[node + 1]]
            if not len(dd):
                continue
            sgn = 1 - 2 * s          # current contribution per edge
            # flipping changes each dst's I by -2*sgn
            delta = int(-4 * sgn * imb[dd].sum() + 4 * len(dd))
            if delta < 0:
                sigma[node] = 1 - s
                counts[c, s] -= 1
                counts[c, 1 - s] += 1
                np.add.at(imb, dd, -2 * sgn)
                flips += 1
        if not flips:
            break
    return sigma


def _host_prep(src, dst, n_nodes, n_cores):
    assert n_nodes % n_cores == 0
    shard_orig = n_nodes // n_cores
    nblk = -(-(shard_orig + 1) // P)
    shard = nblk * P
    np_tot = n_cores * shard

    sigma = _parity_assign(src, dst, n_nodes, n_cores, shard_orig)

    # per-destination parity in-degrees (parity of a source = sigma, fixed
    # regardless of which block/slot it ends up in)
    deg_e = np.zeros(n_nodes, np.int64)
    deg_o = np.zeros(n_nodes, np.int64)
    np.add.at(deg_e, dst[sigma[src] == 0], 1)
    np.add.at(deg_o, dst[sigma[src] == 1], 1)

    # re-bin: per core, sort each class by max(deg_e, deg_o) desc and fill
    # blocks with 64 evens + 64 odds (ghosts absorb exhausted classes)
    perm_new2old = np.full(np_tot, -1, np.int64)
    perm_old2new = np.empty(n_nodes, np.int64)
    key = np.maximum(deg_e, deg_o)
    for c in range(n_cores):
        ids = np.arange(c * shard_orig, (c + 1) * shard_orig)
        ev = ids[sigma[ids] == 0]
        od = ids[sigma[ids] == 1]
        ev = ev[np.argsort(-key[ev], kind="stable")]
        od = od[np.argsort(-key[od], kind="stable")]
        ei = oi = 0
        for b in range(nblk):
            ne = min(P // 2, len(ev) - ei)
            no = min(P // 2, len(od) - oi)
            base = c * shard + b * P
            perm_new2old[base + 0:base + 2 * ne:2] = ev[ei:ei + ne]
            perm_new2old[base + 1:base + 2 * no + 1:2] = od[oi:oi + no]
            ei += ne
            oi += no
        sl = perm_new2old[c * shard:(c + 1) * shard]
        ok = sl >= 0
        perm_old2new[sl[ok]] = c * shard + np.arange(shard)[ok]

    new_src = perm_old2new[src]
    new_dst = perm_old2new[dst]

    # guard rows: every core's ghost slots work (their shard writes set
    # s=-1e9). Padding cycles over all of them — a single guard row would
    # be a same-bank HBM hotspot under 8-core random-read load.
    ghost = np.where(perm_new2old < 0)[0]
    guards = (ghost[ghost % 2 == 0], ghost[ghost % 2 == 1])
    assert len(guards[0]) and len(guards[1])

    Kpar = np.zeros((2, nblk), np.int64)   # [parity, block] max col count
    deg_par = np.zeros((2, np_tot), np.int64)
    par = (new_src & 1).astype(np.int64)
    np.add.at(deg_par, (par, new_dst), 1)
    for pbit in (0, 1):
        dp = deg_par[pbit].reshape(n_cores, nblk, P)
        Kpar[pbit] = np.maximum(dp.max(axis=(0, 2)), 1)

    # slot tables [np_tot, Kmax] per parity, padding spread over all guards
    tables = []
    for pbit in (0, 1):
        kmax = int(Kpar[pbit].max())
        g = guards[pbit]
        fill = np.arange(np_tot * kmax).reshape(np_tot, kmax)
        tab = g[fill % len(g)]
        sel = par == pbit
        sdst = new_dst[sel]
        ssrc = new_src[sel]
        o = np.argsort(sdst, kind="stable")
        sdst = sdst[o]
        ssrc = ssrc[o]
        counts = np.bincount(sdst, minlength=np_tot)
        st = np.concatenate([[0], np.cumsum(counts)[:-1]])
        slot = np.arange(len(sdst)) - st[sdst]
        tab[sdst, slot] = ssrc
        tables.append(tab)

    # idx16 images per core: per block, regions [E cols | O cols],
    # positions k-major (pos = k*128 + p), idx value = new_src >> 1.
    # image[pos % 16, pos // 16] wrapped; replicated to 128 partitions.
    sumK = int(Kpar.sum())
    idx_imgs = np.empty((n_cores, P, sumK * 8), np.int16)
    for c in range(n_cores):
        cols = []
        for b in range(nblk):
            for pbit in (0, 1):
                K = int(Kpar[pbit][b])
                blk = tables[pbit][c * shard + b * P:c * shard + (b + 1) * P, :K]
                pos_val = (blk.T >> 1).reshape(-1)      # k-major
                cols.append(pos_val.reshape(-1, 16).T)  # [16, K*8]
        img16 = np.concatenate(cols, axis=1).astype(np.int16)
        img = np.tile(img16, (8, 1))
        idx_imgs[c] = img

    return dict(
        perm_new2old=perm_new2old,
        perm_old2new=perm_old2new,
        shard=shard,
        shard_orig=shard_orig,
        nblk=nblk,
        np_tot=np_tot,
        Kpar=Kpar,
        sumK=sumK,
        idx_imgs=idx_imgs,
    )


# --------------------------------------------------------------------------
# Device program
# --------------------------------------------------------------------------

def _build_nc(meta, n_cores, in_dim, hid, out_dim, bench_reps=0,
              bench_gather_only=False):
    shard = meta["shard"]
    nblk = meta["nblk"]
    np_tot = meta["np_tot"]
    Kpar = meta["Kpar"]
    shard_orig = meta["shard_orig"]
    sumK = meta["sumK"]
    augc = hid + 2  # h | s | d

    nc = bacc.Bacc(
        "TRN2", target_bir_lowering=False, debug=False,
        num_devices=n_cores, num_swdge_queues=4,
    )

    xin = nc.dram_tensor("x_shard", [shard, in_dim], F32, kind="ExternalInput").ap()
    idx_in = nc.dram_tensor("idx", [P, sumK * 8], I16, kind="ExternalInput").ap()
    gmask_in = nc.dram_tensor("gmask", [P, nblk], F32, kind="ExternalInput").ap()
    ident_in = nc.dram_tensor("ident", [P, P], F32, kind="ExternalInput").ap()
    w1_in = nc.dram_tensor("W1", [in_dim, hid], F32, kind="ExternalInput").ap()
    a1s_in = nc.dram_tensor("a1s", [hid, 1], F32, kind="ExternalInput").ap()
    a1d_in = nc.dram_tensor("a1d", [hid, 1], F32, kind="ExternalInput").ap()
    b1_in = nc.dram_tensor("b1", [1, hid], F32, kind="ExternalInput").ap()
    w2_in = nc.dram_tensor("W2", [hid, out_dim], F32, kind="ExternalInput").ap()
    a2s_in = nc.dram_tensor("a2s", [out_dim, 1], F32, kind="ExternalInput").ap()
    a2d_in = nc.dram_tensor("a2d", [out_dim, 1], F32, kind="ExternalInput").ap()
    b2_in = nc.dram_tensor("b2", [1, out_dim], F32, kind="ExternalInput").ap()
    out_t = nc.dram_tensor("out", [shard, out_dim], F32, kind="ExternalOutput").ap()

    groups = [list(range(n_cores))]
    qctr = [0]

    def next_q():
        q = qctr[0] & 3
        qctr[0] += 1
        return q

    with tile.TileContext(nc) as tc:
        with (
            tc.tile_pool(name="dram", bufs=1, space="DRAM") as dram,
            tc.tile_pool(name="consts", bufs=1) as consts,
            tc.tile_pool(name="sb", bufs=3) as sb,
            tc.tile_pool(name="gather", bufs=6) as gp,
            tc.tile_pool(name="prodp", bufs=5) as prp,
            tc.tile_pool(name="psum", bufs=2, space="PSUM") as pp,
        ):
            shared = "Shared" if n_cores > 1 else "Local"
            hg1 = dram.tile([np_tot, ROWE], BF16, addr_space=shared)
            hg2 = dram.tile([np_tot, ROWE], BF16, addr_space=shared)

            identity = consts.tile([P, P], F32)
            nc.sync.dma_start(out=identity[:], in_=ident_in)
            gmask = consts.tile([P, nblk], F32)
            nc.sync.dma_start(out=gmask[:], in_=gmask_in)

            idx_all = consts.tile([P, sumK * 8], I16)
            nc.sync.dma_start(out=idx_all[:], in_=idx_in)

            def ghost_fix(h_sb, blk):
                if (blk + 1) * P > shard_orig:
                    nc.vector.scalar_tensor_tensor(
                        out=h_sb[:, hid:hid + 1],
                        in0=gmask[:, blk:blk + 1],
                        scalar=NEG_BIG,
                        in1=h_sb[:, hid:hid + 1],
                        op0=mybir.AluOpType.mult,
                        op1=mybir.AluOpType.add,
                    )

            def make_wb(w_in, as_in, ad_in, k, m, tag):
                wb = consts.tile([k, m + 2], F32, tag=f"wb{tag}", name=f"wb{tag}")
                nc.sync.dma_start(out=wb[:, :m], in_=w_in)
                w_sb = sb.tile([k, m], F32, tag="setup_w", name=f"w_sb{tag}")
                nc.sync.dma_start(out=w_sb[:], in_=w_in)
                wT_ps = pp.tile([m, k], F32, tag="xT", name=f"wT_ps{tag}")
                nc.tensor.transpose(out=wT_ps[:], in_=w_sb[:], identity=identity[:k, :k])
                wT_sb = sb.tile([m, k], F32, tag="setup_wT", name=f"wT_sb{tag}")
                nc.vector.tensor_copy(out=wT_sb[:], in_=wT_ps[:])
                av = sb.tile([m, 2], F32, tag="setup_av", name=f"av{tag}")
                nc.sync.dma_start(out=av[:, 0:1], in_=as_in)
                nc.sync.dma_start(out=av[:, 1:2], in_=ad_in)
                v_ps = pp.tile([k, 2], F32, tag="haug", name=f"v_ps{tag}")
                nc.tensor.matmul(
                    out=v_ps[:], lhsT=wT_sb[:], rhs=av[:], start=True, stop=True
                )
                nc.vector.tensor_copy(out=wb[:, m:m + 2], in_=v_ps[:])
                return wb

            wb1 = make_wb(w1_in, a1s_in, a1d_in, in_dim, hid, "1")
            wb2 = make_wb(w2_in, a2s_in, a2d_in, hid, out_dim, "2")

            ones = consts.tile([1, P], F32)
            nc.vector.memset(ones[:], 1.0)

            def make_brep(b_in, m, tag):
                b_sb = sb.tile([1, m], F32, tag="setup_b", name=f"b_sb{tag}")
                nc.sync.dma_start(out=b_sb[:], in_=b_in)
                b_ps = pp.tile([P, m], F32, tag="haug", name=f"b_ps{tag}")
                nc.tensor.matmul(
                    out=b_ps[:], lhsT=ones[:], rhs=b_sb[:], start=True, stop=True
                )
                brep = consts.tile([P, m], F32, tag=f"brep{tag}", name=f"brep{tag}")
                nc.vector.tensor_copy(out=brep[:], in_=b_ps[:])
                return brep

            b1rep = make_brep(b1_in, hid, "1")
            b2rep = make_brep(b2_in, out_dim, "2")

            d1col = consts.tile([P, nblk], F32)
            d2col = consts.tile([P, nblk], F32)

            sh1 = dram.tile([shard, ROWE], BF16)
            sh2 = dram.tile([shard, ROWE], BF16)

            def to_row(h_sb, blk, dcol, sh):
                """Convert haug f32 [P, augc] -> bf16 row and store to sh."""
                nc.vector.tensor_copy(
                    out=dcol[:, blk:blk + 1], in_=h_sb[:, hid + 1:hid + 2]
                )
                ghost_fix(h_sb, blk)
                hrow = sb.tile([P, ROWE], BF16, tag="hrow", name="hrow")
                nc.scalar.copy(out=hrow[:, :hid], in_=h_sb[:, :hid])
                nc.vector.tensor_copy(
                    out=hrow[:, hid:hid + 2].bitcast(F32),
                    in_=h_sb[:, hid:hid + 1],
                )
                nc.sync.dma_start(
                    out=sh[blk * P:(blk + 1) * P, :], in_=hrow[:]
                )

            # ---- phase 1: h1_aug = x_shard @ Wb1 ----------------------
            for t in range(nblk):
                x_t = sb.tile([P, in_dim], F32, tag="x_t")
                nc.sync.dma_start(out=x_t[:], in_=xin[t * P:(t + 1) * P, :])
                xT_ps = pp.tile([in_dim, P], F32, tag="xT")
                nc.tensor.transpose(out=xT_ps[:], in_=x_t[:], identity=identity[:])
                xT_sb = sb.tile([in_dim, P], F32, tag="xT_sb")
                nc.vector.tensor_copy(out=xT_sb[:], in_=xT_ps[:])
                h_ps = pp.tile([P, augc], F32, tag="haug")
                nc.tensor.matmul(
                    out=h_ps[:], lhsT=xT_sb[:], rhs=wb1[:], start=True, stop=True
                )
                h_sb = sb.tile([P, augc], F32, tag="h_sb")
                nc.scalar.copy(out=h_sb[:], in_=h_ps[:])
                to_row(h_sb, t, d1col, sh1)

            if n_cores > 1:
                nc.gpsimd.collective_compute(
                    "AllGather", mybir.AluOpType.bypass,
                    replica_groups=groups, ins=[sh1[:, :]], outs=[hg1[:, :]],
                )
            else:
                nc.sync.dma_start(out=hg1[:, :], in_=sh1[:, :])

            # ---- aggregation ------------------------------------------
            def aggregate(hg, dcol):
                hgv = hg[:].rearrange("(a b) e -> a (b e)", b=2)
                views = (hgv[:, :ROWE], hgv[:, ROWE:])
                icol = 0
                for b in range(nblk):
                    Ke = int(Kpar[0][b])
                    Ko = int(Kpar[1][b])
                    K = Ke + Ko
                    gth = gp.tile([P, K * ROWE], BF16, tag="gth")
                    co = 0
                    for pbit, Kp in ((0, Ke), (1, Ko)):
                        done = 0
                        while done < Kp:
                            nk = min(Kp - done, NG // P)
                            nidx = nk * P
                            nc.gpsimd.dma_gather(
                                out_ap=gth[:, (co + done) * ROWE:
                                           (co + done + nk) * ROWE]
                                    .rearrange("p (j e) -> p j e", e=ROWE),
                                in_ap=views[pbit],
                                idxs_ap=idx_all[:, icol:icol + nidx // 16],
                                num_idxs=nidx,
                                num_idxs_reg=nidx,
                                elem_size=ROWE,
                                elem_step=2 * ROWE,
                                queue_num=next_q(),
                            )
                            icol += nidx // 16
                            done += nk
                        co += Kp
                    if bench_gather_only:
                        dmy = sb.tile([P, 1], BF16, tag="dmy", name="dmy")
                        nc.vector.tensor_copy(out=dmy[:], in_=gth[:, :1])
                        yield None, None, b
                        continue
                    g3 = gth[:].rearrange("p (k e) -> p k e", e=ROWE)
                    sview = g3[:, :, hid:hid + 2].bitcast(F32).squeeze(2)
                    u = sb.tile([P, K], F32, tag="u")
                    nc.vector.tensor_scalar_add(
                        out=u[:], in0=sview, scalar1=dcol[:, b:b + 1]
                    )
                    v = sb.tile([P, K], F32, tag="v")
                    nc.vector.scalar_tensor_tensor(
                        out=v[:], in0=u[:], scalar=SLOPE, in1=u[:],
                        op0=mybir.AluOpType.mult, op1=mybir.AluOpType.max,
                    )
                    w = sb.tile([P, K], F32, tag="w")
                    z = sb.tile([P, 1], F32, tag="z")
                    nc.scalar.activation(
                        out=w[:], in_=v[:],
                        func=mybir.ActivationFunctionType.Exp, accum_out=z[:],
                    )
                    wbc = w[:].unsqueeze(2).broadcast_to([P, K, hid])
                    prod = prp.tile([P, K * hid], F32, tag="prod")
                    p3 = prod[:].rearrange("p (k e) -> p k e", e=hid)
                    nc.vector.tensor_tensor(
                        out=p3, in0=g3[:, :, :hid], in1=wbc,
                        op=mybir.AluOpType.mult,
                    )
                    num = sb.tile([P, hid], F32, tag="num")
                    nc.vector.tensor_reduce(
                        out=num[:], in_=p3.transpose([0, 2, 1]),
                        axis=mybir.AxisListType.X, op=mybir.AluOpType.add,
                    )
                    zc = sb.tile([P, 1], F32, tag="zc")
                    nc.vector.tensor_scalar_max(out=zc[:], in0=z[:], scalar1=1e-30)
                    rz = sb.tile([P, 1], F32, tag="rz")
                    nc.vector.reciprocal(out=rz[:], in_=zc[:])
                    yield num, rz, b

            # ---- phase 2: aggregate layer 1, compute h2_aug -----------
            def phase2():
                for num, rz, b in aggregate(hg1, d1col):
                    if bench_gather_only:
                        hrow = sb.tile([P, ROWE], BF16, tag="hrow", name="hrow")
                        nc.vector.memset(hrow[:], 0.0)
                        nc.sync.dma_start(
                            out=sh2[b * P:(b + 1) * P, :], in_=hrow[:])
                        continue
                    o = sb.tile([P, hid], F32, tag="o1", name="o")
                    nc.vector.scalar_tensor_tensor(
                        out=o[:], in0=num[:], scalar=rz[:], in1=b1rep[:],
                        op0=mybir.AluOpType.mult, op1=mybir.AluOpType.add,
                    )
                    nc.vector.tensor_scalar_max(out=o[:], in0=o[:], scalar1=0.0)
                    oT_ps = pp.tile([hid, P], F32, tag="oT", name="oT_ps")
                    nc.tensor.transpose(out=oT_ps[:], in_=o[:], identity=identity[:])
                    oT_sb = sb.tile([hid, P], F32, tag="oT_sb", name="oT_sb")
                    nc.scalar.copy(out=oT_sb[:], in_=oT_ps[:])
                    h2_ps = pp.tile([P, augc], F32, tag="haug", name="h2_ps")
                    nc.tensor.matmul(
                        out=h2_ps[:], lhsT=oT_sb[:], rhs=wb2[:],
                        start=True, stop=True,
                    )
                    h2_sb = sb.tile([P, augc], F32, tag="h_sb", name="h2_sb")
                    nc.scalar.copy(out=h2_sb[:], in_=h2_ps[:])
                    to_row(h2_sb, b, d2col, sh2)

            if bench_reps:
                with tc.For_i(0, bench_reps, 1):
                    phase2()
            else:
                phase2()

            if n_cores > 1:
                nc.gpsimd.collective_compute(
                    "AllGather", mybir.AluOpType.bypass,
                    replica_groups=groups, ins=[sh2[:, :]], outs=[hg2[:, :]],
                )
            else:
                nc.sync.dma_start(out=hg2[:, :], in_=sh2[:, :])

            # ---- phase 3: aggregate layer 2, normalize, store ---------
            def phase3():
                for num, rz, b in aggregate(hg2, d2col):
                    if bench_gather_only:
                        of = sb.tile([P, out_dim], F32, tag="of", name="of")
                        nc.vector.memset(of[:], 0.0)
                        nc.sync.dma_start(
                            out=out_t[b * P:(b + 1) * P, :], in_=of[:])
                        continue
                    o = sb.tile([P, out_dim], F32, tag="o2", name="o")
                    nc.vector.scalar_tensor_tensor(
                        out=o[:], in0=num[:], scalar=rz[:], in1=b2rep[:],
                        op0=mybir.AluOpType.mult, op1=mybir.AluOpType.add,
                    )
                    sq = sb.tile([P, out_dim], F32, tag="sq", name="sq")
                    ss = sb.tile([P, 1], F32, tag="ss", name="ss")
                    nc.scalar.activation(
                        out=sq[:], in_=o[:],
                        func=mybir.ActivationFunctionType.Square,
                        accum_out=ss[:],
                    )
                    nc.vector.tensor_scalar_max(out=ss[:], in0=ss[:], scalar1=1e-20)
                    lns = sb.tile([P, 1], F32, tag="lns", name="lns")
                    nc.scalar.activation(
                        out=lns[:], in_=ss[:], func=mybir.ActivationFunctionType.Ln
                    )
                    rn = sb.tile([P, 1], F32, tag="rn", name="rn")
                    nc.scalar.activation(
                        out=rn[:], in_=lns[:],
                        func=mybir.ActivationFunctionType.Exp, scale=-0.5,
                    )
                    of = sb.tile([P, out_dim], F32, tag="of", name="of")
                    nc.scalar.mul(out=of[:], in_=o[:], mul=rn[:])
                    nc.sync.dma_start(out=out_t[b * P:(b + 1) * P, :], in_=of[:])

            if bench_reps:
                with tc.For_i(0, bench_reps, 1):
                    phase3()
            else:
                phase3()

    nc.compile()
    return nc


# --------------------------------------------------------------------------
# Entry point
# --------------------------------------------------------------------------

def kernel(
    x, edge_index, W1, att_src1, att_dst1, b1, W2, att_src2, att_dst2, b2,
    _n_cores=8,
):
    global LAST_RESULTS, LAST_META
    x = np.asarray(x, np.float32)
    edge_index = np.asarray(edge_index)
    src = edge_index[0].astype(np.int64)
    dst = edge_index[1].astype(np.int64)
    n_nodes = x.shape[0]
    in_dim = x.shape[1]
    hid = np.asarray(W1).shape[1]
    out_dim = np.asarray(W2).shape[1]

    key = (hash(edge_index.tobytes()), n_nodes, in_dim, hid, out_dim, _n_cores)
    if key in _PROGRAM_CACHE:
        meta, nc = _PROGRAM_CACHE[key]
    else:
        meta = _host_prep(src, dst, n_nodes, _n_cores)
        nc = _build_nc(meta, _n_cores, in_dim, hid, out_dim)
        _PROGRAM_CACHE[key] = (meta, nc)
    LAST_META = meta
    shard = meta["shard"]
    nblk = meta["nblk"]

    common = {
        "ident": np.eye(P, dtype=np.float32),
        "W1": np.ascontiguousarray(W1, np.float32),
        "a1s": np.ascontiguousarray(np.asarray(att_src1, np.float32).reshape(hid, 1)),
        "a1d": np.ascontiguousarray(np.asarray(att_dst1, np.float32).reshape(hid, 1)),
        "b1": np.ascontiguousarray(np.asarray(b1, np.float32).reshape(1, hid)),
        "W2": np.ascontiguousarray(W2, np.float32),
        "a2s": np.ascontiguousarray(np.asarray(att_src2, np.float32).reshape(out_dim, 1)),
        "a2d": np.ascontiguousarray(np.asarray(att_dst2, np.float32).reshape(out_dim, 1)),
        "b2": np.ascontiguousarray(np.asarray(b2, np.float32).reshape(1, out_dim)),
    }
    in_maps = []
    for c in range(_n_cores):
        ids = meta["perm_new2old"][c * shard:(c + 1) * shard]
        x_shard = np.zeros((shard, in_dim), np.float32)
        real = ids >= 0
        x_shard[real] = x[ids[real]]
        gmask = (ids < 0).reshape(nblk, P).T.astype(np.float32)
        in_maps.append(dict(
            common, x_shard=x_shard, idx=meta["idx_imgs"][c],
            gmask=np.ascontiguousarray(gmask),
        ))

    res = bass_utils.run_bass_kernel_spmd(
        nc, in_maps, core_ids=list(range(_n_cores))
    )
    LAST_RESULTS = res

    full = np.empty((n_nodes, out_dim), np.float32)
    for c in range(_n_cores):
        ids = meta["perm_new2old"][c * shard:(c + 1) * shard]
        real = ids >= 0
        full[ids[real]] = res.results[c]["out"][real]
    return full


# revision 19
# speedup vs baseline: 428.8386x; 1.0565x over previous
"""Two-layer GAT (single-head, PyG-style) on 8 Trainium2 NeuronCores — v2.

Strategy (destination-sharded, as v1) with a rebuilt gather pipeline:
  - Table rows are 256B bf16 (h[64] bf16 | s f32 in 2 bf16 slots | pad)
    instead of 512B f32: random-access HBM reads are ~3.4x faster per row
    at 256B, and bf16 h is well within the 2e-2 tolerance (s stays f32).
  - Per-edge rows are fetched with batched SWDGE dma_gather (<=1024 rows
    per instruction, round-robined over 4 SWDGE queues) instead of one
    qPoolDynamic indirect DMA per slot column: ~10x fewer Pool-engine
    instructions and ~3x more DMA-queue parallelism.
  - dma_gather indices are int16 (<=32767) but the table has 50176 rows:
    slots are split by *source-id parity* and fetched from even/odd
    strided views of the table (elem_step=512B, index = row >> 1, max
    25088). A host-side greedy discrepancy pass chooses which nodes get
    even/odd ids (within each 128-node block, 64/64) so that each
    destination's in-edges split ~evenly and per-block slot counts stay
    near ceil(K/2) per parity.
  - The table AllGather moves bf16 rows (half the bytes of v1), and
    padding slots cycle over every core's ghost rows: a single guard row
    would serialize ~100k same-address HBM reads on one bank (measured
    ~6x slowdown of the whole aggregation phase).
"""

import numpy as np

import concourse.bacc as bacc
import concourse.bass as bass
import concourse.mybir as mybir
import concourse.tile as tile
from concourse import bass_utils

F32 = mybir.dt.float32
BF16 = mybir.dt.bfloat16
I16 = mybir.dt.int16
P = 128
ROWE = 128          # table row = 128 bf16 = 256B
SLOPE = 0.2
NEG_BIG = -1.0e9
NG = 1024           # max rows per dma_gather (SWDGE ring limit)

LAST_RESULTS = None
LAST_META = None

_PROGRAM_CACHE = {}


# --------------------------------------------------------------------------
# Host-side preprocessing
# --------------------------------------------------------------------------

def _parity_assign(src, dst, n_nodes, n_cores, shard_orig):
    """Greedy discrepancy: pick ~half of each core's nodes for even ids so
    each destination's in-edges split evenly between even and odd sources.

    Budget: per core at most ceil((shard_orig+pad)/2) per class (block
    re-binning later needs 64/64 per block, ghosts absorb the remainder).
    Returns sigma[old_id] in {0 (even), 1}.
    """
    o = np.argsort(src, kind="stable")
    odst = dst[o]
    starts = np.searchsorted(src[o], np.arange(n_nodes + 1))
    imb = np.zeros(n_nodes, np.int64)
    sigma = np.full(n_nodes, -1, np.int8)
    deg = np.bincount(dst, minlength=n_nodes)
    nblk = -(-(shard_orig + 1) // P)
    half = (nblk * P) // 2 - 1   # leave >=1 ghost slot per parity (guards)
    counts = np.zeros((n_cores, 2), np.int64)
    # pass 0: quadratic greedy (sigma = -sign(sum of dst imbalances))
    for c in range(n_cores):
        ids = np.arange(c * shard_orig, (c + 1) * shard_orig)
        order = np.argsort(-deg[ids], kind="stable")
        budget = [half, half]
        for node in ids[order]:
            dd = odst[starts[node]:starts[node + 1]]
            t = int(imb[dd].sum()) if len(dd) else 0
            s = 0 if (t <= 0) else 1
            if budget[s] == 0:
                s = 1 - s
            sigma[node] = s
            budget[s] -= 1
            if len(dd):
                np.add.at(imb, dd, 1 - 2 * s)
        counts[c, 0] = half - budget[0]
        counts[c, 1] = half - budget[1]
    # improvement passes: flip a node when it lowers sum(I^2) and budgets
    # stay legal. delta(flip) = sum over dsts of ((I -+ 2)^2 - I^2).
    for _ in range(3):
        flips = 0
        for node in range(n_nodes):
            c = node // shard_orig
            s = int(sigma[node])
            if counts[c, 1 - s] >= half:
                continue
            dd = odst[starts[node]:starts[node + 1]]
            if not len(dd):
                continue
            sgn = 1 - 2 * s          # current contribution per edge
            # flipping changes each dst's I by -2*sgn
            delta = int(-4 * sgn * imb[dd].sum() + 4 * len(dd))
            if delta < 0:
                sigma[node] = 1 - s
                counts[c, s] -= 1
                counts[c, 1 - s] += 1
                np.add.at(imb, dd, -2 * sgn)
                flips += 1
        if not flips:
            break
    return sigma


def _host_prep(src, dst, n_nodes, n_cores):
    assert n_nodes % n_cores == 0
    shard_orig = n_nodes // n_cores
    nblk = -(-(shard_orig + 1) // P)
    shard = nblk * P
    np_tot = n_cores * shard

    sigma = _parity_assign(src, dst, n_nodes, n_cores, shard_orig)

    # per-destination parity in-degrees (parity of a source = sigma, fixed
    # regardless of which block/slot it ends up in)
    deg_e = np.zeros(n_nodes, np.int64)
    deg_o = np.zeros(n_nodes, np.int64)
    np.add.at(deg_e, dst[sigma[src] == 0], 1)
    np.add.at(deg_o, dst[sigma[src] == 1], 1)

    # re-bin: per core, sort each class by max(deg_e, deg_o) desc and fill
    # blocks with 64 evens + 64 odds (ghosts absorb exhausted classes)
    perm_new2old = np.full(np_tot, -1, np.int64)
    perm_old2new = np.empty(n_nodes, np.int64)
    key = np.maximum(deg_e, deg_o)
    for c in range(n_cores):
        ids = np.arange(c * shard_orig, (c + 1) * shard_orig)
        ev = ids[sigma[ids] == 0]
        od = ids[sigma[ids] == 1]
        ev = ev[np.argsort(-key[ev], kind="stable")]
        od = od[np.argsort(-key[od], kind="stable")]
        ei = oi = 0
        for b in range(nblk):
            ne = min(P // 2, len(ev) - ei)
            no = min(P // 2, len(od) - oi)
            base = c * shard + b * P
            perm_new2old[base + 0:base + 2 * ne:2] = ev[ei:ei + ne]
            perm_new2old[base + 1:base + 2 * no + 1:2] = od[oi:oi + no]
            ei += ne
            oi += no
        sl = perm_new2old[c * shard:(c + 1) * shard]
        ok = sl >= 0
        perm_old2new[sl[ok]] = c * shard + np.arange(shard)[ok]

    new_src = perm_old2new[src]
    new_dst = perm_old2new[dst]

    # guard rows: every core's ghost slots work (their shard writes set
    # s=-1e9). Padding cycles over all of them — a single guard row would
    # be a same-bank HBM hotspot under 8-core random-read load.
    ghost = np.where(perm_new2old < 0)[0]
    guards = (ghost[ghost % 2 == 0], ghost[ghost % 2 == 1])
    assert len(guards[0]) and len(guards[1])

    Kpar = np.zeros((2, nblk), np.int64)   # [parity, block] max col count
    deg_par = np.zeros((2, np_tot), np.int64)
    par = (new_src & 1).astype(np.int64)
    np.add.at(deg_par, (par, new_dst), 1)
    for pbit in (0, 1):
        dp = deg_par[pbit].reshape(n_cores, nblk, P)
        Kpar[pbit] = np.maximum(dp.max(axis=(0, 2)), 1)

    # slot tables [np_tot, Kmax] per parity, padding spread over all guards
    tables = []
    for pbit in (0, 1):
        kmax = int(Kpar[pbit].max())
        g = guards[pbit]
        fill = np.arange(np_tot * kmax).reshape(np_tot, kmax)
        tab = g[fill % len(g)]
        sel = par == pbit
        sdst = new_dst[sel]
        ssrc = new_src[sel]
        o = np.argsort(sdst, kind="stable")
        sdst = sdst[o]
        ssrc = ssrc[o]
        counts = np.bincount(sdst, minlength=np_tot)
        st = np.concatenate([[0], np.cumsum(counts)[:-1]])
        slot = np.arange(len(sdst)) - st[sdst]
        tab[sdst, slot] = ssrc
        tables.append(tab)

    # idx16 images per core: per block, regions [E cols | O cols],
    # positions k-major (pos = k*128 + p), idx value = new_src >> 1.
    # image[pos % 16, pos // 16] wrapped; replicated to 128 partitions.
    sumK = int(Kpar.sum())
    idx_imgs = np.empty((n_cores, P, sumK * 8), np.int16)
    for c in range(n_cores):
        cols = []
        for b in range(nblk):
            for pbit in (0, 1):
                K = int(Kpar[pbit][b])
                blk = tables[pbit][c * shard + b * P:c * shard + (b + 1) * P, :K]
                pos_val = (blk.T >> 1).reshape(-1)      # k-major
                cols.append(pos_val.reshape(-1, 16).T)  # [16, K*8]
        img16 = np.concatenate(cols, axis=1).astype(np.int16)
        img = np.tile(img16, (8, 1))
        idx_imgs[c] = img

    return dict(
        perm_new2old=perm_new2old,
        perm_old2new=perm_old2new,
        shard=shard,
        shard_orig=shard_orig,
        nblk=nblk,
        np_tot=np_tot,
        Kpar=Kpar,
        sumK=sumK,
        idx_imgs=idx_imgs,
    )


# --------------------------------------------------------------------------
# Device program
# --------------------------------------------------------------------------

def _build_nc(meta, n_cores, in_dim, hid, out_dim, bench_reps=0,
              bench_gather_only=False):
    shard = meta["shard"]
    nblk = meta["nblk"]
    np_tot = meta["np_tot"]
    Kpar = meta["Kpar"]
    shard_orig = meta["shard_orig"]
    sumK = meta["sumK"]
    augc = hid + 2  # h | s | d

    nc = bacc.Bacc(
        "TRN2", target_bir_lowering=False, debug=False,
        num_devices=n_cores, num_swdge_queues=4,
    )

    xin = nc.dram_tensor("x_shard", [shard, in_dim], F32, kind="ExternalInput").ap()
    idx_in = nc.dram_tensor("idx", [P, sumK * 8], I16, kind="ExternalInput").ap()
    gmask_in = nc.dram_tensor("gmask", [P, nblk], F32, kind="ExternalInput").ap()
    ident_in = nc.dram_tensor("ident", [P, P], F32, kind="ExternalInput").ap()
    w1_in = nc.dram_tensor("W1", [in_dim, hid], F32, kind="ExternalInput").ap()
    a1s_in = nc.dram_tensor("a1s", [hid, 1], F32, kind="ExternalInput").ap()
    a1d_in = nc.dram_tensor("a1d", [hid, 1], F32, kind="ExternalInput").ap()
    b1_in = nc.dram_tensor("b1", [1, hid], F32, kind="ExternalInput").ap()
    w2_in = nc.dram_tensor("W2", [hid, out_dim], F32, kind="ExternalInput").ap()
    a2s_in = nc.dram_tensor("a2s", [out_dim, 1], F32, kind="ExternalInput").ap()
    a2d_in = nc.dram_tensor("a2d", [out_dim, 1], F32, kind="ExternalInput").ap()
    b2_in = nc.dram_tensor("b2", [1, out_dim], F32, kind="ExternalInput").ap()
    out_t = nc.dram_tensor("out", [shard, out_dim], F32, kind="ExternalOutput").ap()

    groups = [list(range(n_cores))]
    qctr = [0]

    def next_q():
        q = qctr[0] & 3
        qctr[0] += 1
        return q

    with tile.TileContext(nc) as tc:
        with (
            tc.tile_pool(name="dram", bufs=1, space="DRAM") as dram,
            tc.tile_pool(name="consts", bufs=1) as consts,
            tc.tile_pool(name="sb", bufs=3) as sb,
            tc.tile_pool(name="gather", bufs=6) as gp,
            tc.tile_pool(name="prodp", bufs=5) as prp,
            tc.tile_pool(name="psum", bufs=2, space="PSUM") as pp,
        ):
            shared = "Shared" if n_cores > 1 else "Local"
            hg1 = dram.tile([np_tot, ROWE], BF16, addr_space=shared)
            hg2 = dram.tile([np_tot, ROWE], BF16, addr_space=shared)

            identity = consts.tile([P, P], F32)
            nc.sync.dma_start(out=identity[:], in_=ident_in)
            gmask = consts.tile([P, nblk], F32)
            nc.sync.dma_start(out=gmask[:], in_=gmask_in)

            idx_all = consts.tile([P, sumK * 8], I16)
            nc.sync.dma_start(out=idx_all[:], in_=idx_in)

            def ghost_fix(h_sb, blk):
                if (blk + 1) * P > shard_orig:
                    nc.vector.scalar_tensor_tensor(
                        out=h_sb[:, hid:hid + 1],
                        in0=gmask[:, blk:blk + 1],
                        scalar=NEG_BIG,
                        in1=h_sb[:, hid:hid + 1],
                        op0=mybir.AluOpType.mult,
                        op1=mybir.AluOpType.add,
                    )

            def make_wb(w_in, as_in, ad_in, k, m, tag):
                wb = consts.tile([k, m + 2], F32, tag=f"wb{tag}", name=f"wb{tag}")
                nc.sync.dma_start(out=wb[:, :m], in_=w_in)
                w_sb = sb.tile([k, m], F32, tag="setup_w", name=f"w_sb{tag}")
                nc.sync.dma_start(out=w_sb[:], in_=w_in)
                wT_ps = pp.tile([m, k], F32, tag="xT", name=f"wT_ps{tag}")
                nc.tensor.transpose(out=wT_ps[:], in_=w_sb[:], identity=identity[:k, :k])
                wT_sb = sb.tile([m, k], F32, tag="setup_wT", name=f"wT_sb{tag}")
                nc.vector.tensor_copy(out=wT_sb[:], in_=wT_ps[:])
                av = sb.tile([m, 2], F32, tag="setup_av", name=f"av{tag}")
                nc.sync.dma_start(out=av[:, 0:1], in_=as_in)
                nc.sync.dma_start(out=av[:, 1:2], in_=ad_in)
                v_ps = pp.tile([k, 2], F32, tag="haug", name=f"v_ps{tag}")
                nc.tensor.matmul(
                    out=v_ps[:], lhsT=wT_sb[:], rhs=av[:], start=True, stop=True
                )
                nc.vector.tensor_copy(out=wb[:, m:m + 2], in_=v_ps[:])
                return wb

            wb1 = make_wb(w1_in, a1s_in, a1d_in, in_dim, hid, "1")
            wb2 = make_wb(w2_in, a2s_in, a2d_in, hid, out_dim, "2")

            ones = consts.tile([1, P], F32)
            nc.vector.memset(ones[:], 1.0)

            def make_brep(b_in, m, tag):
                b_sb = sb.tile([1, m], F32, tag="setup_b", name=f"b_sb{tag}")
                nc.sync.dma_start(out=b_sb[:], in_=b_in)
                b_ps = pp.tile([P, m], F32, tag="haug", name=f"b_ps{tag}")
                nc.tensor.matmul(
                    out=b_ps[:], lhsT=ones[:], rhs=b_sb[:], start=True, stop=True
                )
                brep = consts.tile([P, m], F32, tag=f"brep{tag}", name=f"brep{tag}")
                nc.vector.tensor_copy(out=brep[:], in_=b_ps[:])
                return brep

            b1rep = make_brep(b1_in, hid, "1")
            b2rep = make_brep(b2_in, out_dim, "2")

            d1col = consts.tile([P, nblk], F32)
            d2col = consts.tile([P, nblk], F32)

            sh1 = dram.tile([shard, ROWE], BF16)
            sh2 = dram.tile([shard, ROWE], BF16)

            def to_row(h_sb, blk, dcol, sh):
                """Convert haug f32 [P, augc] -> bf16 row and store to sh."""
                nc.vector.tensor_copy(
                    out=dcol[:, blk:blk + 1], in_=h_sb[:, hid + 1:hid + 2]
                )
                ghost_fix(h_sb, blk)
                hrow = sb.tile([P, ROWE], BF16, tag="hrow", name="hrow")
                nc.scalar.copy(out=hrow[:, :hid], in_=h_sb[:, :hid])
                nc.vector.tensor_copy(
                    out=hrow[:, hid:hid + 2].bitcast(F32),
                    in_=h_sb[:, hid:hid + 1],
                )
                nc.sync.dma_start(
                    out=sh[blk * P:(blk + 1) * P, :], in_=hrow[:]
                )

            # ---- phase 1: h1_aug = x_shard @ Wb1 ----------------------
            for t in range(nblk):
                x_t = sb.tile([P, in_dim], F32, tag="x_t")
                nc.sync.dma_start(out=x_t[:], in_=xin[t * P:(t + 1) * P, :])
                xT_ps = pp.tile([in_dim, P], F32, tag="xT")
                nc.tensor.transpose(out=xT_ps[:], in_=x_t[:], identity=identity[:])
                xT_sb = sb.tile([in_dim, P], F32, tag="xT_sb")
                nc.vector.tensor_copy(out=xT_sb[:], in_=xT_ps[:])
                h_ps = pp.tile([P, augc], F32, tag="haug")
                nc.tensor.matmul(
                    out=h_ps[:], lhsT=xT_sb[:], rhs=wb1[:], start=True, stop=True
                )
                h_sb = sb.tile([P, augc], F32, tag="h_sb")
                nc.scalar.copy(out=h_sb[:], in_=h_ps[:])
                to_row(h_sb, t, d1col, sh1)

            if n_cores > 1:
                nc.gpsimd.collective_compute(
                    "AllGather", mybir.AluOpType.bypass,
                    replica_groups=groups, ins=[sh1[:, :]], outs=[hg1[:, :]],
                )
            else:
                nc.sync.dma_start(out=hg1[:, :], in_=sh1[:, :])

            # ---- aggregation ------------------------------------------
            def aggregate(hg, dcol):
                hgv = hg[:].rearrange("(a b) e -> a (b e)", b=2)
                views = (hgv[:, :ROWE], hgv[:, ROWE:])
                icol = 0
                for b in range(nblk):
                    Ke = int(Kpar[0][b])
                    Ko = int(Kpar[1][b])
                    K = Ke + Ko
                    gth = gp.tile([P, K * ROWE], BF16, tag="gth")
                    co = 0
                    for pbit, Kp in ((0, Ke), (1, Ko)):
                        nch = -(-Kp // (NG // P))
                        bnk, extra = divmod(Kp, nch)
                        done = 0
                        for ci in range(nch):
                            nk = bnk + (1 if ci < extra else 0)
                            nidx = nk * P
                            nc.gpsimd.dma_gather(
                                out_ap=gth[:, (co + done) * ROWE:
                                           (co + done + nk) * ROWE]
                                    .rearrange("p (j e) -> p j e", e=ROWE),
                                in_ap=views[pbit],
                                idxs_ap=idx_all[:, icol:icol + nidx // 16],
                                num_idxs=nidx,
                                num_idxs_reg=nidx,
                                elem_size=ROWE,
                                elem_step=2 * ROWE,
                                queue_num=next_q(),
                            )
                            icol += nidx // 16
                            done += nk
                        co += Kp
                    if bench_gather_only:
                        dmy = sb.tile([P, 1], BF16, tag="dmy", name="dmy")
                        nc.vector.tensor_copy(out=dmy[:], in_=gth[:, :1])
                        yield None, None, b
                        continue
                    g3 = gth[:].rearrange("p (k e) -> p k e", e=ROWE)
                    sview = g3[:, :, hid:hid + 2].bitcast(F32).squeeze(2)
                    u = sb.tile([P, K], F32, tag="u")
                    nc.vector.tensor_scalar_add(
                        out=u[:], in0=sview, scalar1=dcol[:, b:b + 1]
                    )
                    v = sb.tile([P, K], F32, tag="v")
                    nc.vector.scalar_tensor_tensor(
                        out=v[:], in0=u[:], scalar=SLOPE, in1=u[:],
                        op0=mybir.AluOpType.mult, op1=mybir.AluOpType.max,
                    )
                    w = sb.tile([P, K], F32, tag="w")
                    z = sb.tile([P, 1], F32, tag="z")
                    nc.scalar.activation(
                        out=w[:], in_=v[:],
                        func=mybir.ActivationFunctionType.Exp, accum_out=z[:],
                    )
                    wbc = w[:].unsqueeze(2).broadcast_to([P, K, hid])
                    prod = prp.tile([P, K * hid], F32, tag="prod")
                    p3 = prod[:].rearrange("p (k e) -> p k e", e=hid)
                    nc.vector.tensor_tensor(
                        out=p3, in0=g3[:, :, :hid], in1=wbc,
                        op=mybir.AluOpType.mult,
                    )
                    num = sb.tile([P, hid], F32, tag="num")
                    nc.vector.tensor_reduce(
                        out=num[:], in_=p3.transpose([0, 2, 1]),
                        axis=mybir.AxisListType.X, op=mybir.AluOpType.add,
                    )
                    zc = sb.tile([P, 1], F32, tag="zc")
                    nc.vector.tensor_scalar_max(out=zc[:], in0=z[:], scalar1=1e-30)
                    rz = sb.tile([P, 1], F32, tag="rz")
                    nc.vector.reciprocal(out=rz[:], in_=zc[:])
                    yield num, rz, b

            # ---- phase 2: aggregate layer 1, compute h2_aug -----------
            def phase2():
                for num, rz, b in aggregate(hg1, d1col):
                    if bench_gather_only:
                        hrow = sb.tile([P, ROWE], BF16, tag="hrow", name="hrow")
                        nc.vector.memset(hrow[:], 0.0)
                        nc.sync.dma_start(
                            out=sh2[b * P:(b + 1) * P, :], in_=hrow[:])
                        continue
                    o = sb.tile([P, hid], F32, tag="o1", name="o")
                    nc.vector.scalar_tensor_tensor(
                        out=o[:], in0=num[:], scalar=rz[:], in1=b1rep[:],
                        op0=mybir.AluOpType.mult, op1=mybir.AluOpType.add,
                    )
                    nc.vector.tensor_scalar_max(out=o[:], in0=o[:], scalar1=0.0)
                    oT_ps = pp.tile([hid, P], F32, tag="oT", name="oT_ps")
                    nc.tensor.transpose(out=oT_ps[:], in_=o[:], identity=identity[:])
                    oT_sb = sb.tile([hid, P], F32, tag="oT_sb", name="oT_sb")
                    nc.scalar.copy(out=oT_sb[:], in_=oT_ps[:])
                    h2_ps = pp.tile([P, augc], F32, tag="haug", name="h2_ps")
                    nc.tensor.matmul(
                        out=h2_ps[:], lhsT=oT_sb[:], rhs=wb2[:],
                        start=True, stop=True,
                    )
                    h2_sb = sb.tile([P, augc], F32, tag="h_sb", name="h2_sb")
                    nc.scalar.copy(out=h2_sb[:], in_=h2_ps[:])
                    to_row(h2_sb, b, d2col, sh2)

            if bench_reps:
                with tc.For_i(0, bench_reps, 1):
                    phase2()
            else:
                phase2()

            if n_cores > 1:
                nc.gpsimd.collective_compute(
                    "AllGather", mybir.AluOpType.bypass,
                    replica_groups=groups, ins=[sh2[:, :]], outs=[hg2[:, :]],
                )
            else:
                nc.sync.dma_start(out=hg2[:, :], in_=sh2[:, :])

            # ---- phase 3: aggregate layer 2, normalize, store ---------
            def phase3():
                for num, rz, b in aggregate(hg2, d2col):
                    if bench_gather_only:
                        of = sb.tile([P, out_dim], F32, tag="of", name="of")
                        nc.vector.memset(of[:], 0.0)
                        nc.sync.dma_start(
                            out=out_t[b * P:(b + 1) * P, :], in_=of[:])
                        continue
                    o = sb.tile([P, out_dim], F32, tag="o2", name="o")
                    nc.vector.scalar_tensor_tensor(
                        out=o[:], in0=num[:], scalar=rz[:], in1=b2rep[:],
                        op0=mybir.AluOpType.mult, op1=mybir.AluOpType.add,
                    )
                    sq = sb.tile([P, out_dim], F32, tag="sq", name="sq")
                    ss = sb.tile([P, 1], F32, tag="ss", name="ss")
                    nc.scalar.activation(
                        out=sq[:], in_=o[:],
                        func=mybir.ActivationFunctionType.Square,
                        accum_out=ss[:],
                    )
                    nc.vector.tensor_scalar_max(out=ss[:], in0=ss[:], scalar1=1e-20)
                    lns = sb.tile([P, 1], F32, tag="lns", name="lns")
                    nc.scalar.activation(
                        out=lns[:], in_=ss[:], func=mybir.ActivationFunctionType.Ln
                    )
                    rn = sb.tile([P, 1], F32, tag="rn", name="rn")
                    nc.scalar.activation(
                        out=rn[:], in_=lns[:],
                        func=mybir.ActivationFunctionType.Exp, scale=-0.5,
                    )
                    of = sb.tile([P, out_dim], F32, tag="of", name="of")
                    nc.scalar.mul(out=of[:], in_=o[:], mul=rn[:])
                    nc.sync.dma_start(out=out_t[b * P:(b + 1) * P, :], in_=of[:])

            if bench_reps:
                with tc.For_i(0, bench_reps, 1):
                    phase3()
            else:
                phase3()

    nc.compile()
    return nc


# --------------------------------------------------------------------------
# Entry point
# --------------------------------------------------------------------------

def kernel(
    x, edge_index, W1, att_src1, att_dst1, b1, W2, att_src2, att_dst2, b2,
    _n_cores=8,
):
    global LAST_RESULTS, LAST_META
    x = np.asarray(x, np.float32)
    edge_index = np.asarray(edge_index)
    src = edge_index[0].astype(np.int64)
    dst = edge_index[1].astype(np.int64)
    n_nodes = x.shape[0]
    in_dim = x.shape[1]
    hid = np.asarray(W1).shape[1]
    out_dim = np.asarray(W2).shape[1]

    key = (hash(edge_index.tobytes()), n_nodes, in_dim, hid, out_dim, _n_cores)
    if key in _PROGRAM_CACHE:
        meta, nc = _PROGRAM_CACHE[key]
    else:
        meta = _host_prep(src, dst, n_nodes, _n_cores)
        nc = _build_nc(meta, _n_cores, in_dim, hid, out_dim)
        _PROGRAM_CACHE[key] = (meta, nc)
    LAST_META = meta
    shard = meta["shard"]
    nblk = meta["nblk"]

    common = {
        "ident": np.eye(P, dtype=np.float32),
        "W1": np.ascontiguousarray(W1, np.float32),
        "a1s": np.ascontiguousarray(np.asarray(att_src1, np.float32).reshape(hid, 1)),
        "a1d": np.ascontiguousarray(np.asarray(att_dst1, np.float32).reshape(hid, 1)),
        "b1": np.ascontiguousarray(np.asarray(b1, np.float32).reshape(1, hid)),
        "W2": np.ascontiguousarray(W2, np.float32),
        "a2s": np.ascontiguousarray(np.asarray(att_src2, np.float32).reshape(out_dim, 1)),
        "a2d": np.ascontiguousarray(np.asarray(att_dst2, np.float32).reshape(out_dim, 1)),
        "b2": np.ascontiguousarray(np.asarray(b2, np.float32).reshape(1, out_dim)),
    }
    in_maps = []
    for c in range(_n_cores):
        ids = meta["perm_new2old"][c * shard:(c + 1) * shard]
        x_shard = np.zeros((shard, in_dim), np.float32)
        real = ids >= 0
        x_shard[real] = x[ids[real]]
        gmask = (ids < 0).reshape(nblk, P).T.astype(np.float32)
        in_maps.append(dict(
            common, x_shard=x_shard, idx=meta["idx_imgs"][c],
            gmask=np.ascontiguousarray(gmask),
        ))

    res = bass_utils.run_bass_kernel_spmd(
        nc, in_maps, core_ids=list(range(_n_cores))
    )
    LAST_RESULTS = res

    full = np.empty((n_nodes, out_dim), np.float32)
    for c in range(_n_cores):
        ids = meta["perm_new2old"][c * shard:(c + 1) * shard]
        real = ids >= 0
        full[ids[real]] = res.results[c]["out"][real]
    return full
